# revision 51
# baseline (speedup 1.0000x reference)
"""Causal MHA (B=4, L=2048, D=1024, H=16) on 8 NeuronCores.

Sharding: core c -> (batch b = c//2, head-group g = c%2). Data-parallel over
the 4 batches, tensor-parallel over heads (8 heads per core): wq/wk/wv
column-parallel, wo row-parallel. Each core returns a partial [L, D] output;
the host sums the two head-group partials per batch and adds wo_b.

Single fused streaming kernel, no DRAM round-trips:
  A(n):  Q/K/V projections in bf16 (x/w bf16, psum f32, Q/K kept f32r in
         SBUF for exact S logits). Emitted as ~0.9us units interleaved
         between B(n-1) heads to keep the PE busy while ACT runs exp.
  B(n):  per head: S.T[keys,q] = KT_h.T @ QT_h (f32r, causal-trimmed to
         N>=256), exp on ACT with full-history kb blocks PAIRED into
         [128,2,512] psum tiles (halves ACT per-instruction overhead) ->
         pt bf16; tri-mask on the diagonal tile (DVE); flipped AV:
         avps[q, 4t, 65] += pt_blk.T @ vaug (bf16, ones column = softmax
         denominator landing on the partition axis) -> per-partition
         reciprocal + tensor_scalar_mul normalize (no broadcasts).
  T(n):  ctxn [q,512] -> ctxT [d,q] via xbar DMA-transpose (off the PE);
         the LAST slice uses PE transposes + DVE copy instead (PE is idle
         at the tail and this cuts the xbar DMA latency off the critical
         path).
  C(n):  out[tok,1024] = sum_c ctxT[c].T @ wo[c] (bf16), drained 2-per-head
         into the ACT-bound later slices on the psA psum ring; the tail
         writes per-512-column halves for faster outs-ring recycle.
PSUM budget (8 banks): S pairs 2x2 + AV accum 2 (unserializes consecutive
heads) + shared A-proj/C/transpose ring 2.
"""

import numpy as np
import ml_dtypes

import concourse.bacc as bacc
import concourse.bass as bass
import concourse.mybir as mybir
import concourse.tile as tile
from concourse.bass_utils import run_bass_kernel_spmd

F32 = mybir.dt.float32
F32R = mybir.dt.float32r
BF16 = mybir.dt.bfloat16
F8 = mybir.dt.float8e4
DR = mybir.MatmulPerfMode.DoubleRow

B, L, D, H, DK = 4, 2048, 1024, 16, 64
HD = 8             # heads per core
GW = 512           # head-group width (8 heads * 64)
AUGW = HD * (DK + 1)   # 520: per head 64 dims + ones col (ones LAST per head)
NCH = D // 128     # 8 contraction chunks
NSL = 4            # token slices of 512
NTT = L // 128     # 16 token tiles
WS = 1.0           # V-path pre-scale (cancels in softmax ratio; 1 for bf16)
ESC = 0.125        # exp scale: 1/sqrt(DK)
EBI = -2.0         # exp bias: shift-invariant headroom so exp fits fp8e4m3


def _build_nc(dbg=False):
    import os
    LOOKAHEAD = bool(int(os.environ.get("KCFG_LOOKAHEAD", "0")))
    ARATE = int(os.environ.get("KCFG_ARATE", "2"))
    CRATES = [int(x) for x in os.environ.get("KCFG_CRATE", "2,2,2,2").split(",")]
    INHEAD = bool(int(os.environ.get("KCFG_INHEAD", "0")))
    BIASACT = bool(int(os.environ.get("KCFG_BIASACT", "0")))
    WEAVE = bool(int(os.environ.get("KCFG_WEAVE", "0")))
    nc = bacc.Bacc("TRN2", target_bir_lowering=False, debug=False, num_devices=8)

    xq = nc.dram_tensor("xq", [128, NCH, L], BF16, kind="ExternalInput").ap()
    xk = nc.dram_tensor("xk", [128, NCH, L], BF16, kind="ExternalInput").ap()
    xv = nc.dram_tensor("xv", [128, NCH, L], BF16, kind="ExternalInput").ap()
    wq = nc.dram_tensor("wq", [128, NCH, GW], BF16, kind="ExternalInput").ap()
    wk = nc.dram_tensor("wk", [128, NCH, GW], BF16, kind="ExternalInput").ap()
    wv = nc.dram_tensor("wv", [128, NCH, AUGW], BF16, kind="ExternalInput").ap()
    wo = nc.dram_tensor("wo", [128, 4, D], BF16, kind="ExternalInput").ap()
    bq = nc.dram_tensor("bq", [128, 4], F32, kind="ExternalInput").ap()
    bk = nc.dram_tensor("bk", [128, 4], F32, kind="ExternalInput").ap()
    vb = nc.dram_tensor("vb", [AUGW], F32, kind="ExternalInput").ap()
    msk = nc.dram_tensor("msk", [128, 128], BF16, kind="ExternalInput").ap()
    idn = nc.dram_tensor("idn", [128, 128], BF16, kind="ExternalInput").ap()
    outp = nc.dram_tensor("outp", [L, D], F32, kind="ExternalOutput").ap()
    if dbg:
        qt_dbg = nc.dram_tensor("qt_dbg", [128, 4, GW], F32,
                                kind="ExternalOutput").ap()
        kt_dbg = nc.dram_tensor("kt_dbg", [128, 4, L], F32,
                                kind="ExternalOutput").ap()
        vg_dbg = nc.dram_tensor("vg_dbg", [128, NTT, AUGW], BF16,
                                kind="ExternalOutput").ap()
        cn_dbg = nc.dram_tensor("cn_dbg", [128, NTT, GW], BF16,
                                kind="ExternalOutput").ap()
        ct_dbg = nc.dram_tensor("ct_dbg", [128, NTT, 4, 128], BF16,
                                kind="ExternalOutput").ap()

    with tile.TileContext(nc) as tc:
        with (
            tc.tile_pool(name="persist", bufs=1) as persist,
            tc.tile_pool(name="qtp", bufs=(4 if dbg else 3)) as qtp,
            tc.tile_pool(name="xqk", bufs=(3 if dbg else 4)) as xqkp,
            tc.tile_pool(name="xvp", bufs=2) as xvp,
            tc.tile_pool(name="ptp", bufs=(3 if dbg else 5)) as ptp,
            tc.tile_pool(name="ctxn", bufs=(16 if dbg else 6)) as ctxnp,
            tc.tile_pool(name="ctxT", bufs=16) as ctxTp,
            tc.tile_pool(name="rcp", bufs=4) as rcp,
            tc.tile_pool(name="outs", bufs=(2 if dbg else 4)) as outsp,
            tc.tile_pool(name="psS", bufs=2, space="PSUM") as psS,
            tc.tile_pool(name="psAV", bufs=2, space="PSUM") as psAV,
            tc.tile_pool(name="psA", bufs=2, space="PSUM") as psA,
        ):
            # ---- persistent SBUF ----
            wq_s = persist.tile([128, NCH, GW], BF16, tag="wq")
            wk_s = persist.tile([128, NCH, GW], BF16, tag="wk")
            wv_s = persist.tile([128, NCH, AUGW], BF16, tag="wv")
            wo_s = persist.tile([128, 4, D], BF16, tag="wo")
            kt_s = persist.tile([128, 4, L], F32R, tag="kt")
            vaug_s = persist.tile([128, NTT, AUGW], BF16, tag="vaug")
            bq_s = persist.tile([128, 4], F32, tag="bq")
            bk_s = persist.tile([128, 4], F32, tag="bk")
            vb_s = persist.tile([128, AUGW], BF16, tag="vb")
            msk_s = persist.tile([128, 128], BF16, tag="msk")
            idn_s = persist.tile([128, 128], BF16, tag="idn")

            # weight/const loads; order = DMA engine order (startup latency)
            xq_tiles = {}
            xk_tiles = {}
            xv_tiles = {}

            def issue_xin(n):
                c0, c1 = n * 512, (n + 1) * 512
                t = xqkp.tile([128, NCH, 512], BF16, tag="xqk", name=f"xq{n}")
                nc.sync.dma_start(t[:, :, :], xq[:, :, c0:c1])
                xq_tiles[n] = t
                t = xqkp.tile([128, NCH, 512], BF16, tag="xqk", name=f"xk{n}")
                nc.sync.dma_start(t[:, :, :], xk[:, :, c0:c1])
                xk_tiles[n] = t
                t = xvp.tile([128, NCH, 512], BF16, tag="xv", name=f"xv{n}")
                nc.sync.dma_start(t[:, :, :], xv[:, :, c0:c1])
                xv_tiles[n] = t

            # startup order matches phase-A(0) consumption: alternate
            # Q/K weight+input quarters so the first 4 QK units stream in
            tq = xqkp.tile([128, NCH, 512], BF16, tag="xqk", name="xq0")
            tk = xqkp.tile([128, NCH, 512], BF16, tag="xqk", name="xk0")
            nc.sync.dma_start(wq_s[:, :, 0:256], wq[:, :, 0:256])
            nc.sync.dma_start(tq[:, :, 0:256], xq[:, :, 0:256])
            nc.sync.dma_start(wk_s[:, :, 0:256], wk[:, :, 0:256])
            nc.sync.dma_start(tk[:, :, 0:256], xk[:, :, 0:256])
            nc.sync.dma_start(bq_s[:, :], bq[:, :])
            nc.sync.dma_start(bk_s[:, :], bk[:, :])
            nc.sync.dma_start(wq_s[:, :, 256:512], wq[:, :, 256:512])
            nc.sync.dma_start(tq[:, :, 256:512], xq[:, :, 256:512])
            nc.sync.dma_start(wk_s[:, :, 256:512], wk[:, :, 256:512])
            nc.sync.dma_start(tk[:, :, 256:512], xk[:, :, 256:512])
            xq_tiles[0] = tq
            xk_tiles[0] = tk
            nc.sync.dma_start(wv_s[:, :, :], wv[:, :, :])
            t0 = xvp.tile([128, NCH, 512], BF16, tag="xv", name="xv0")
            nc.sync.dma_start(t0[:, :, :], xv[:, :, 0:512])
            xv_tiles[0] = t0
            vb_bcast = bass.AP(tensor=vb.tensor, offset=vb.offset,
                               ap=[[0, 128], [1, AUGW]])
            nc.gpsimd.dma_start(vb_s[:, :], vb_bcast)
            nc.sync.dma_start(msk_s[:, :], msk[:, :])
            nc.sync.dma_start(idn_s[:, :], idn[:, :])
            nc.sync.dma_start(wo_s[:, :, :], wo[:, :, :])

            qt_tiles = {}

            # ---- phase A unit generator: fp8 DoubleRow projections ----
            def a_units(n):
                qt_t = qtp.tile([128, 4, GW], F32R, tag="qt", name=f"qt{n}")
                qt_tiles[n] = qt_t

                def qk_unit(hf, g, x_of, w_s, b_s, is_q):
                    # split into two ~0.9us halves (one mi each) so fillers
                    # can weave between S/exp steps without starving ACT
                    ps_box = []

                    def half(mi):
                        def emit():
                            x_t = x_of[n]
                            if mi == 0:
                                ps_box.append(psA.tile(
                                    [128, 2, 256], F32, tag="pa",
                                    name=f"pa{n}_{hf}_{g}"))
                            ps = ps_box[0]
                            for c in range(NCH):
                                # start=True zeroes the whole psum bank:
                                # only the first write into the tile sets it
                                nc.tensor.matmul(
                                    ps[:, mi, :],
                                    w_s[:, c, (2 * g + mi) * 128:
                                        (2 * g + mi + 1) * 128],
                                    x_t[:, c, hf * 256:hf * 256 + 256],
                                    start=(c == 0 and mi == 0),
                                    stop=(c == NCH - 1),
                                    skip_group_check=True)
                            m = 2 * g + mi
                            if is_q:
                                dst = qt_t[:, m, hf * 256:hf * 256 + 256]
                            else:
                                dst = kt_s[:, m, n * 512 + hf * 256:
                                           n * 512 + hf * 256 + 256]
                            if BIASACT:
                                nc.scalar.activation(
                                    dst, ps[:, mi, :],
                                    func=mybir.ActivationFunctionType.Identity,
                                    bias=b_s[:, m:m + 1])
                            else:
                                nc.vector.tensor_scalar_add(
                                    dst, ps[:, mi, :], b_s[:, m:m + 1])
                        return emit
                    return [half(0), half(1)]

                def v_unit(tt, vhf):
                    def emit():
                        ps = psA.tile([128, 260], F32, tag="pa",
                                      name=f"pv{n}_{tt}_{vhf}")
                        xv_t = xv_tiles[n]
                        for c in range(NCH):
                            nc.tensor.matmul(
                                ps[:, :],
                                xv_t[:, c, tt * 128:(tt + 1) * 128],
                                wv_s[:, c, vhf * 260:(vhf + 1) * 260],
                                start=(c == 0), stop=(c == NCH - 1))
                        nc.vector.tensor_add(
                            vaug_s[:, n * 4 + tt, vhf * 260:(vhf + 1) * 260],
                            ps[:, :], vb_s[:, vhf * 260:(vhf + 1) * 260])
                    return emit

                units = []
                for hf in range(2):
                    for g in range(2):
                        units.extend(qk_unit(hf, g, xq_tiles, wq_s, bq_s, True))
                        units.extend(qk_unit(hf, g, xk_tiles, wk_s, bk_s,
                                             False))
                for hf in range(2):
                    for tt in (2 * hf, 2 * hf + 1):
                        for vhf in range(2):
                            units.append(v_unit(tt, vhf))
                return units

            ctxn_tiles = {}
            ctxT_tiles = {}

            # ---- phase B: one head of slice n ----
            # returns (s_emitters, av_emitters, finalize) so the slice loop
            # can weave the next head's first S blocks before this head's
            # tail, keeping ACT fed across head boundaries
            def plan_head(n, h):
                po = (h % 2) * 64
                mc = h // 2
                qt_t = qt_tiles[n]
                nkb = 4 * n + 4
                avps = psAV.tile([128, 4, DK + 1], F32, tag="av",
                                 name=f"av{n}_{h}")
                # S/exp units: full-history kb pairs, then 2 diag pairs
                pt_of = {}   # kb -> (tile, region)
                ptm_of = {}  # kb -> masked diag tile
                sunits = [("pair", p) for p in range(2 * n)]
                sunits += [("dpair", 0), ("dpair", 1)]

                def emit_s(u):
                    kind, a = u
                    sp = psS.tile([128, 2, 512], F32, tag="sp",
                                  name=f"sp{n}_{h}_{kind}{a}")
                    if kind == "pair":
                        for i in range(2):
                            kb = 2 * a + i
                            # regions 0/1 are in different banks: each needs
                            # its own start=True (bank-granular zeroing)
                            nc.tensor.matmul(
                                sp[:, i, :],
                                kt_s[po:po + 64, mc, kb * 128:(kb + 1) * 128],
                                qt_t[po:po + 64, mc, :],
                                start=True, stop=True,
                                skip_group_check=True)
                        pt = ptp.tile([128, 2, 512], BF16, tag="pt",
                                      name=f"pt{n}_{h}_p{a}")
                        nc.scalar.activation(
                            pt[:, :, :], sp[:, :, :],
                            func=mybir.ActivationFunctionType.Exp, scale=ESC)
                        pt_of[2 * a] = (pt, 0)
                        pt_of[2 * a + 1] = (pt, 1)
                    else:
                        # diagonal pair d: blocks jj = 2d, 2d+1 share one
                        # 2-bank psum tile and ONE exp; unwritten psum cols
                        # are zeroed by start=True (exp(0*s)=1, never read)
                        d = a
                        col0x = 256 * d
                        for i in range(2):
                            kb = 4 * n + 2 * d + i
                            jj = 2 * d + i
                            col0s = min(jj * 128, 256)
                            nc.tensor.matmul(
                                sp[:, i, col0s:],
                                kt_s[po:po + 64, mc, kb * 128:(kb + 1) * 128],
                                qt_t[po:po + 64, mc, col0s:],
                                start=True, stop=True, skip_group_check=True)
                        pt = ptp.tile([128, 2, 512], BF16, tag="pt",
                                      name=f"pt{n}_{h}_dp{d}")
                        nc.scalar.activation(
                            pt[:, :, col0x:], sp[:, :, col0x:],
                            func=mybir.ActivationFunctionType.Exp, scale=ESC)
                        for i in range(2):
                            jj = 2 * d + i
                            # out-of-place mask: unmasked consumers (j > jj)
                            # read pt directly without waiting on DVE
                            ptm = ptp.tile([128, 128], BF16, tag="ptm",
                                           name=f"ptm{n}_{h}_{2 * d + i}",
                                           bufs=4)
                            nc.vector.tensor_mul(
                                ptm[:, :],
                                pt[:, i, jj * 128:(jj + 1) * 128], msk_s[:, :])
                            pt_of[4 * n + jj] = (pt, i)
                            ptm_of[4 * n + jj] = ptm

                def emit_av(u):
                    kind, a = u
                    if kind == "pair":
                        kbs = [2 * a, 2 * a + 1]
                    else:
                        kbs = [4 * n + 2 * a, 4 * n + 2 * a + 1]
                    for kb in kbs:
                        j0 = max(0, kb - 4 * n)
                        pt, reg = pt_of[kb]
                        for j in range(j0, 4):
                            if j == kb - 4 * n:
                                lhs = ptm_of[kb][:, :]
                            else:
                                lhs = pt[:, reg, j * 128:(j + 1) * 128]
                            # whole-bank zero on start: only first mm sets it
                            nc.tensor.matmul(
                                avps[:, j, :], lhs,
                                vaug_s[:, kb, h * 65:(h + 1) * 65],
                                start=(kb == 0 and j == 0),
                                stop=(kb == 4 * n + j),
                                skip_group_check=True)

                def finalize():
                    rc = rcp.tile([128, 4], F32, tag="rc", name=f"rc{n}_{h}")
                    nc.vector.reciprocal(rc[:, :], avps[:, :, 64])
                    for j in range(4):
                        nc.vector.tensor_scalar_mul(
                            ctxn_tiles[(n, j)][:, h * 64:(h + 1) * 64],
                            avps[:, j, 0:64], rc[:, j:j + 1])

                s_emit = [(lambda u: (lambda: emit_s(u)))(u) for u in sunits]
                av_emit = [(lambda u: (lambda: emit_av(u)))(u) for u in sunits]
                return s_emit, av_emit, finalize

            # ---- phase C unit: token tile t, output half n2 ----
            out_tiles = {}

            def c_unit(n, j, n2):
                t = 4 * n + j
                # C units share the psA ring (A units are gone or sparse
                # by the time C drains); psC's bank went to psAV=2 which
                # unserializes consecutive heads' AV accumulation
                pool = psA
                ptag = "pa"

                def emit():
                    if n2 == 0 and n < NSL - 1:
                        out_tiles[t] = outsp.tile([128, D], F32, tag="outs",
                                                  name=f"out{t}")
                    cps = pool.tile([128, 512], F32, tag=ptag,
                                    name=f"cps{t}_{n2}")
                    ctxT_t = ctxT_tiles[(n, j)]
                    for c in range(4):
                        nc.tensor.matmul(
                            cps[:, :], ctxT_t[:, c, :],
                            wo_s[:, c, n2 * 512:(n2 + 1) * 512],
                            start=(c == 0), stop=(c == 3))
                    if n == NSL - 1:
                        # tail: per-half copy + immediate DMA (faster outs
                        # ring recycle than full-row tiles)
                        oh = outsp.tile([128, 512], F32, tag="outs",
                                        name=f"outh{t}_{n2}")
                        nc.vector.tensor_copy(oh[:, :], cps[:, :])
                        nc.sync.dma_start(
                            outp[t * 128:(t + 1) * 128,
                                 n2 * 512:(n2 + 1) * 512], oh[:, :])
                    else:
                        nc.vector.tensor_copy(
                            out_tiles[t][:, n2 * 512:(n2 + 1) * 512],
                            cps[:, :])
                        if n2 == 1:
                            nc.sync.dma_start(
                                outp[t * 128:(t + 1) * 128, :],
                                out_tiles[t][:, :])
                return emit

            # ---- main schedule ----
            for u in a_units(0):
                u()

            a_queue = []          # (slice, unit) in slice order
            pending_c = []
            c_rate = dict(enumerate(CRATES))

            for n in range(NSL):
                if LOOKAHEAD:
                    if n == 0:
                        issue_xin(1)
                        issue_xin(2)
                        for u in a_units(1):
                            a_queue.append((1, u))
                        for u in a_units(2):
                            a_queue.append((2, u))
                    elif n == 1:
                        issue_xin(3)
                        for u in a_units(3):
                            a_queue.append((3, u))
                else:
                    if n < NSL - 1:
                        issue_xin(n + 1)
                        for u in a_units(n + 1):
                            a_queue.append((n + 1, u))
                for j in range(4):
                    ctxn_tiles[(n, j)] = ctxnp.tile(
                        [128, GW], BF16, tag="ctxn", name=f"ctxn{n}_{j}")
                fillq = []
                for _ in range(ARATE * HD):
                    if a_queue:
                        fillq.append(a_queue.pop(0)[1])
                for _ in range(c_rate.get(n, 2) * HD):
                    if pending_c:
                        fillq.append(pending_c.pop(0))
                if WEAVE:
                    prev_fin = None
                    for h in range(HD):
                        s_emit, av_emit, fin = plan_head(n, h)
                        ns_ = len(s_emit)
                        s_emit[0]()
                        if prev_fin is not None:
                            prev_fin()
                        s_emit[1]()
                        if fillq:
                            fillq.pop(0)()
                        for i in range(2, ns_):
                            s_emit[i]()
                            av_emit[i - 2]()
                            if fillq:
                                fillq.pop(0)()
                        av_emit[ns_ - 2]()
                        av_emit[ns_ - 1]()
                        prev_fin = fin
                    prev_fin()
                    while fillq:
                        fillq.pop(0)()
                else:
                    nfill = len(fillq)
                    for h in range(HD):
                        s_emit, av_emit, fin = plan_head(n, h)
                        ns_ = len(s_emit)
                        s_emit[0]()
                        if ns_ > 1:
                            s_emit[1]()
                        for i in range(2, ns_):
                            s_emit[i]()
                            av_emit[i - 2]()
                        av_emit[ns_ - 2]()
                        av_emit[ns_ - 1]()
                        fin()
                        take = (nfill * (h + 1)) // HD - (nfill * h) // HD
                        for _ in range(take):
                            if fillq:
                                fillq.pop(0)()
                # B(n+1) needs all of A(n+1) done
                while a_queue and a_queue[0][0] <= n + 1:
                    a_queue.pop(0)[1]()
                if n < NSL - 1:
                    for j in range(4):
                        ct = ctxTp.tile([128, 4, 128], BF16, tag="ctxT",
                                        name=f"ctxT{n}_{j}")
                        nc.sync.dma_start_transpose(ct,
                                                    ctxn_tiles[(n, j)][:, :])
                        ctxT_tiles[(n, j)] = ct
                    for j in range(4):
                        for n2 in range(2):
                            pending_c.append(c_unit(n, j, n2))
                else:
                    # tail: PE transposes (PE is idle here) + DVE copy cut
                    # the ~2.8us-per-tile xbar DMA latency off the critical
                    # path; interleave each transpose with its C unit
                    for j in range(4):
                        tp = psA.tile([128, 4, 128], BF16, tag="pa",
                                      name=f"tp{j}")
                        cn_t = ctxn_tiles[(n, j)]
                        for c in range(4):
                            nc.tensor.transpose(tp[:, c, :],
                                                cn_t[:, c * 128:(c + 1) * 128],
                                                idn_s[:, :])
                        ct = ctxTp.tile([128, 4, 128], BF16, tag="ctxT",
                                        name=f"ctxT{n}_{j}")
                        nc.vector.tensor_copy(ct[:, :, :], tp[:, :, :])
                        ctxT_tiles[(n, j)] = ct
                        pending_c.append(c_unit(n, j, 0))
                        pending_c.append(c_unit(n, j, 1))
            while pending_c:
                pending_c.pop(0)()

            if dbg:
                nc.sync.dma_start(qt_dbg[:, :, :],
                                  qt_tiles[0][:, :, :].bitcast(F32))
                nc.sync.dma_start(kt_dbg[:, :, :], kt_s[:, :, :].bitcast(F32))
                nc.sync.dma_start(vg_dbg[:, :, :], vaug_s[:, :, :])
                for n in range(NSL):
                    for j in range(4):
                        nc.sync.dma_start(cn_dbg[:, 4 * n + j, :],
                                          ctxn_tiles[(n, j)][:, :])
                        nc.sync.dma_start(ct_dbg[:, 4 * n + j, :, :],
                                          ctxT_tiles[(n, j)][:, :, :])

    nc.compile()
    return nc


_NC = None
LAST_RESULTS = None


def kernel(**inputs):
    global _NC, LAST_RESULTS
    import os
    if _NC is None:
        _NC = _build_nc()

    f = lambda a: np.asarray(a, dtype=np.float32)
    q, k, v = f(inputs["q"]), f(inputs["k"]), f(inputs["v"])
    wq_w, wq_b = f(inputs["wq_w"]), f(inputs["wq_b"])
    wk_w, wk_b = f(inputs["wk_w"]), f(inputs["wk_b"])
    wv_w, wv_b = f(inputs["wv_w"]), f(inputs["wv_b"])
    wo_w, wo_b = f(inputs["wo_w"]), f(inputs["wo_b"])

    bf = ml_dtypes.bfloat16
    f8 = ml_dtypes.float8_e4m3

    def chunk_rows(a, inner):
        # [1024, X] -> [128, 8, X] with row r = c*128+p -> [p, c, :]
        return np.ascontiguousarray(
            a.reshape(NCH, 128, inner).transpose(1, 0, 2))

    msk = np.ascontiguousarray(
        (np.arange(128)[None, :] >= np.arange(128)[:, None])).astype(bf)

    gmaps = []
    for g in range(2):
        sl = slice(g * GW, (g + 1) * GW)
        wqT = chunk_rows(wq_w[sl].T, GW).astype(bf)
        wkT = chunk_rows(wk_w[sl].T, GW).astype(bf)
        wvT = np.zeros((D, AUGW), np.float32)
        vbias = np.zeros((AUGW,), np.float32)
        for h in range(HD):
            wvT[:, h * 65:h * 65 + 64] = wv_w[g * GW + h * 64:
                                              g * GW + (h + 1) * 64].T * WS
            vbias[h * 65:h * 65 + 64] = wv_b[g * GW + h * 64:
                                             g * GW + (h + 1) * 64] * WS
            vbias[h * 65 + 64] = WS
        woT = np.ascontiguousarray(
            wo_w[:, sl].T.reshape(4, 128, D).transpose(1, 0, 2)).astype(bf)
        bqT = np.ascontiguousarray(wq_b[sl].reshape(4, 128).T)
        bkT = np.ascontiguousarray(wk_b[sl].reshape(4, 128).T)
        gmaps.append(dict(wq=wqT, wk=wkT, wv=chunk_rows(wvT, AUGW).astype(bf),
                          wo=woT, bq=bqT, bk=bkT, vb=vbias, msk=msk,
                          idn=np.eye(128, dtype=np.float32).astype(bf)))

    bmaps = []
    for b in range(B):
        bmaps.append(dict(
            xq=chunk_rows(np.ascontiguousarray(q[b].T), L).astype(bf),
            xk=chunk_rows(np.ascontiguousarray(k[b].T), L).astype(bf),
            xv=chunk_rows(np.ascontiguousarray(v[b].T), L).astype(bf)))

    in_maps = [dict(**bmaps[c // 2], **gmaps[c % 2]) for c in range(8)]

    trace = bool(int(os.environ.get("KERNEL_TRACE", "0")))
    res = run_bass_kernel_spmd(_NC, in_maps, list(range(8)), trace=trace)
    LAST_RESULTS = res

    out = np.empty((B, L, D), np.float32)
    for b in range(B):
        out[b] = (res.results[2 * b]["outp"] + res.results[2 * b + 1]["outp"]
                  + wo_b[None, :])
    return out


# revision 52
# speedup vs baseline: 1.0053x; 1.0053x over previous
"""Causal MHA (B=4, L=2048, D=1024, H=16) on 8 NeuronCores.

Sharding: core c -> (batch b = c//2, head-group g = c%2). Data-parallel over
the 4 batches, tensor-parallel over heads (8 heads per core): wq/wk/wv
column-parallel, wo row-parallel. Each core returns a partial [L, D] output;
the host sums the two head-group partials per batch and adds wo_b.

Single fused streaming kernel, no DRAM round-trips:
  A(n):  Q/K/V projections in bf16 (x/w bf16, psum f32, Q/K kept f32r in
         SBUF for exact S logits). Emitted as ~0.9us units interleaved
         between B(n-1) heads to keep the PE busy while ACT runs exp.
  B(n):  per head: S.T[keys,q] = KT_h.T @ QT_h (f32r, causal-trimmed to
         N>=256), exp on ACT with full-history kb blocks PAIRED into
         [128,2,512] psum tiles (halves ACT per-instruction overhead) ->
         pt bf16; tri-mask on the diagonal tile (DVE); flipped AV:
         avps[q, 4t, 65] += pt_blk.T @ vaug (bf16, ones column = softmax
         denominator landing on the partition axis) -> per-partition
         reciprocal + tensor_scalar_mul normalize (no broadcasts).
  T(n):  ctxn [q,512] -> ctxT [d,q] via xbar DMA-transpose (off the PE);
         the LAST slice uses PE transposes + DVE copy instead (PE is idle
         at the tail and this cuts the xbar DMA latency off the critical
         path).
  C(n):  out[tok,1024] = sum_c ctxT[c].T @ wo[c] (bf16), drained 2-per-head
         into the ACT-bound later slices on the psA psum ring; the tail
         writes per-512-column halves for faster outs-ring recycle.
PSUM budget (8 banks): S pairs 2x2 + AV accum 2 (unserializes consecutive
heads) + shared A-proj/C/transpose ring 2.
"""

import numpy as np
import ml_dtypes

import concourse.bacc as bacc
import concourse.bass as bass
import concourse.mybir as mybir
import concourse.tile as tile
from concourse.bass_utils import run_bass_kernel_spmd

F32 = mybir.dt.float32
F32R = mybir.dt.float32r
BF16 = mybir.dt.bfloat16
F8 = mybir.dt.float8e4
DR = mybir.MatmulPerfMode.DoubleRow

B, L, D, H, DK = 4, 2048, 1024, 16, 64
HD = 8             # heads per core
GW = 512           # head-group width (8 heads * 64)
AUGW = HD * (DK + 1)   # 520: per head 64 dims + ones col (ones LAST per head)
NCH = D // 128     # 8 contraction chunks
NSL = 4            # token slices of 512
NTT = L // 128     # 16 token tiles
WS = 1.0           # V-path pre-scale (cancels in softmax ratio; 1 for bf16)
ESC = 0.125        # exp scale: 1/sqrt(DK)
EBI = -2.0         # exp bias: shift-invariant headroom so exp fits fp8e4m3


def _build_nc(dbg=False):
    import os
    LOOKAHEAD = bool(int(os.environ.get("KCFG_LOOKAHEAD", "0")))
    ARATE = int(os.environ.get("KCFG_ARATE", "2"))
    CRATES = [int(x) for x in os.environ.get("KCFG_CRATE", "2,2,2,2").split(",")]
    INHEAD = bool(int(os.environ.get("KCFG_INHEAD", "0")))
    BIASACT = bool(int(os.environ.get("KCFG_BIASACT", "0")))
    WEAVE = bool(int(os.environ.get("KCFG_WEAVE", "0")))
    nc = bacc.Bacc("TRN2", target_bir_lowering=False, debug=False, num_devices=8)

    xq = nc.dram_tensor("xq", [128, NCH, L], BF16, kind="ExternalInput").ap()
    xk = nc.dram_tensor("xk", [128, NCH, L], BF16, kind="ExternalInput").ap()
    xv = nc.dram_tensor("xv", [128, NCH, L], BF16, kind="ExternalInput").ap()
    wq = nc.dram_tensor("wq", [128, NCH, GW], BF16, kind="ExternalInput").ap()
    wk = nc.dram_tensor("wk", [128, NCH, GW], BF16, kind="ExternalInput").ap()
    wv = nc.dram_tensor("wv", [128, NCH, AUGW], BF16, kind="ExternalInput").ap()
    wo = nc.dram_tensor("wo", [128, 4, D], BF16, kind="ExternalInput").ap()
    bq = nc.dram_tensor("bq", [128, 4], F32, kind="ExternalInput").ap()
    bk = nc.dram_tensor("bk", [128, 4], F32, kind="ExternalInput").ap()
    vb = nc.dram_tensor("vb", [AUGW], F32, kind="ExternalInput").ap()
    msk = nc.dram_tensor("msk", [128, 128], BF16, kind="ExternalInput").ap()
    idn = nc.dram_tensor("idn", [128, 128], BF16, kind="ExternalInput").ap()
    outp = nc.dram_tensor("outp", [L, D], F32, kind="ExternalOutput").ap()
    if dbg:
        qt_dbg = nc.dram_tensor("qt_dbg", [128, 4, GW], F32,
                                kind="ExternalOutput").ap()
        kt_dbg = nc.dram_tensor("kt_dbg", [128, 4, L], F32,
                                kind="ExternalOutput").ap()
        vg_dbg = nc.dram_tensor("vg_dbg", [128, NTT, AUGW], BF16,
                                kind="ExternalOutput").ap()
        cn_dbg = nc.dram_tensor("cn_dbg", [128, NTT, GW], BF16,
                                kind="ExternalOutput").ap()
        ct_dbg = nc.dram_tensor("ct_dbg", [128, NTT, 4, 128], BF16,
                                kind="ExternalOutput").ap()

    with tile.TileContext(nc) as tc:
        with (
            tc.tile_pool(name="persist", bufs=1) as persist,
            tc.tile_pool(name="qtp", bufs=(4 if dbg else 3)) as qtp,
            tc.tile_pool(name="xqk", bufs=(3 if dbg else 4)) as xqkp,
            tc.tile_pool(name="xvp", bufs=2) as xvp,
            tc.tile_pool(name="ptp", bufs=(3 if dbg else 5)) as ptp,
            tc.tile_pool(name="ctxn", bufs=(16 if dbg else 6)) as ctxnp,
            tc.tile_pool(name="ctxT", bufs=16) as ctxTp,
            tc.tile_pool(name="rcp", bufs=4) as rcp,
            tc.tile_pool(name="outs", bufs=(2 if dbg else 4)) as outsp,
            tc.tile_pool(name="psS", bufs=2, space="PSUM") as psS,
            tc.tile_pool(name="psAV", bufs=2, space="PSUM") as psAV,
            tc.tile_pool(name="psA", bufs=2, space="PSUM") as psA,
        ):
            # ---- persistent SBUF ----
            wq_s = persist.tile([128, NCH, GW], BF16, tag="wq")
            wk_s = persist.tile([128, NCH, GW], BF16, tag="wk")
            wv_s = persist.tile([128, NCH, AUGW], BF16, tag="wv")
            wo_s = persist.tile([128, 4, D], BF16, tag="wo")
            kt_s = persist.tile([128, 4, L], F32R, tag="kt")
            vaug_s = persist.tile([128, NTT, AUGW], BF16, tag="vaug")
            bq_s = persist.tile([128, 4], F32, tag="bq")
            bk_s = persist.tile([128, 4], F32, tag="bk")
            vb_s = persist.tile([128, AUGW], BF16, tag="vb")
            msk_s = persist.tile([128, 128], BF16, tag="msk")
            idn_s = persist.tile([128, 128], BF16, tag="idn")

            # weight/const loads; order = DMA engine order (startup latency)
            xq_tiles = {}
            xk_tiles = {}
            xv_tiles = {}

            def issue_xin(n):
                c0, c1 = n * 512, (n + 1) * 512
                t = xqkp.tile([128, NCH, 512], BF16, tag="xqk", name=f"xq{n}")
                nc.sync.dma_start(t[:, :, :], xq[:, :, c0:c1])
                xq_tiles[n] = t
                t = xqkp.tile([128, NCH, 512], BF16, tag="xqk", name=f"xk{n}")
                nc.sync.dma_start(t[:, :, :], xk[:, :, c0:c1])
                xk_tiles[n] = t
                t = xvp.tile([128, NCH, 512], BF16, tag="xv", name=f"xv{n}")
                nc.sync.dma_start(t[:, :, :], xv[:, :, c0:c1])
                xv_tiles[n] = t

            # startup order matches phase-A(0) consumption: alternate
            # Q/K weight+input quarters so the first 4 QK units stream in
            tq = xqkp.tile([128, NCH, 512], BF16, tag="xqk", name="xq0")
            tk = xqkp.tile([128, NCH, 512], BF16, tag="xqk", name="xk0")
            nc.sync.dma_start(wq_s[:, :, 0:256], wq[:, :, 0:256])
            nc.sync.dma_start(tq[:, :, 0:256], xq[:, :, 0:256])
            nc.sync.dma_start(wk_s[:, :, 0:256], wk[:, :, 0:256])
            nc.sync.dma_start(tk[:, :, 0:256], xk[:, :, 0:256])
            nc.sync.dma_start(bq_s[:, :], bq[:, :])
            nc.sync.dma_start(bk_s[:, :], bk[:, :])
            nc.sync.dma_start(wq_s[:, :, 256:512], wq[:, :, 256:512])
            nc.sync.dma_start(tq[:, :, 256:512], xq[:, :, 256:512])
            nc.sync.dma_start(wk_s[:, :, 256:512], wk[:, :, 256:512])
            nc.sync.dma_start(tk[:, :, 256:512], xk[:, :, 256:512])
            xq_tiles[0] = tq
            xk_tiles[0] = tk
            nc.sync.dma_start(wv_s[:, :, :], wv[:, :, :])
            t0 = xvp.tile([128, NCH, 512], BF16, tag="xv", name="xv0")
            nc.sync.dma_start(t0[:, :, :], xv[:, :, 0:512])
            xv_tiles[0] = t0
            vb_bcast = bass.AP(tensor=vb.tensor, offset=vb.offset,
                               ap=[[0, 128], [1, AUGW]])
            nc.gpsimd.dma_start(vb_s[:, :], vb_bcast)
            nc.sync.dma_start(msk_s[:, :], msk[:, :])
            nc.sync.dma_start(idn_s[:, :], idn[:, :])
            nc.sync.dma_start(wo_s[:, :, :], wo[:, :, :])

            qt_tiles = {}

            # ---- phase A unit generator: fp8 DoubleRow projections ----
            def a_units(n):
                qt_t = qtp.tile([128, 4, GW], F32R, tag="qt", name=f"qt{n}")
                qt_tiles[n] = qt_t

                def qk_unit(hf, g, x_of, w_s, b_s, is_q):
                    # split into two ~0.9us halves (one mi each) so fillers
                    # can weave between S/exp steps without starving ACT
                    ps_box = []

                    def half(mi):
                        def emit():
                            x_t = x_of[n]
                            if mi == 0:
                                ps_box.append(psA.tile(
                                    [128, 2, 256], F32, tag="pa",
                                    name=f"pa{n}_{hf}_{g}"))
                            ps = ps_box[0]
                            for c in range(NCH):
                                # start=True zeroes the whole psum bank:
                                # only the first write into the tile sets it
                                nc.tensor.matmul(
                                    ps[:, mi, :],
                                    w_s[:, c, (2 * g + mi) * 128:
                                        (2 * g + mi + 1) * 128],
                                    x_t[:, c, hf * 256:hf * 256 + 256],
                                    start=(c == 0 and mi == 0),
                                    stop=(c == NCH - 1),
                                    skip_group_check=True)
                            m = 2 * g + mi
                            if is_q:
                                dst = qt_t[:, m, hf * 256:hf * 256 + 256]
                            else:
                                dst = kt_s[:, m, n * 512 + hf * 256:
                                           n * 512 + hf * 256 + 256]
                            if BIASACT:
                                nc.scalar.activation(
                                    dst, ps[:, mi, :],
                                    func=mybir.ActivationFunctionType.Identity,
                                    bias=b_s[:, m:m + 1])
                            else:
                                nc.vector.tensor_scalar_add(
                                    dst, ps[:, mi, :], b_s[:, m:m + 1])
                        return emit
                    return [half(0), half(1)]

                def v_unit(tt, vhf):
                    def emit():
                        ps = psA.tile([128, 260], F32, tag="pa",
                                      name=f"pv{n}_{tt}_{vhf}")
                        xv_t = xv_tiles[n]
                        for c in range(NCH):
                            nc.tensor.matmul(
                                ps[:, :],
                                xv_t[:, c, tt * 128:(tt + 1) * 128],
                                wv_s[:, c, vhf * 260:(vhf + 1) * 260],
                                start=(c == 0), stop=(c == NCH - 1))
                        nc.vector.tensor_add(
                            vaug_s[:, n * 4 + tt, vhf * 260:(vhf + 1) * 260],
                            ps[:, :], vb_s[:, vhf * 260:(vhf + 1) * 260])
                    return emit

                units = []
                for hf in range(2):
                    for g in range(2):
                        units.extend(qk_unit(hf, g, xq_tiles, wq_s, bq_s, True))
                        units.extend(qk_unit(hf, g, xk_tiles, wk_s, bk_s,
                                             False))
                for hf in range(2):
                    for tt in (2 * hf, 2 * hf + 1):
                        for vhf in range(2):
                            units.append(v_unit(tt, vhf))
                return units

            ctxn_tiles = {}
            ctxT_tiles = {}

            # ---- phase B: one head of slice n ----
            # returns (s_emitters, av_emitters, finalize) so the slice loop
            # can weave the next head's first S blocks before this head's
            # tail, keeping ACT fed across head boundaries
            def plan_head(n, h):
                po = (h % 2) * 64
                mc = h // 2
                qt_t = qt_tiles[n]
                nkb = 4 * n + 4
                avps = psAV.tile([128, 4, DK + 1], F32, tag="av",
                                 name=f"av{n}_{h}")
                # S/exp units: full-history kb pairs, then 2 diag pairs
                pt_of = {}   # kb -> (tile, region)
                ptm_of = {}  # kb -> masked diag tile
                sunits = [("pair", p) for p in range(2 * n)]
                sunits += [("dpair", 0), ("dpair", 1)]

                def emit_s(u):
                    kind, a = u
                    sp = psS.tile([128, 2, 512], F32, tag="sp",
                                  name=f"sp{n}_{h}_{kind}{a}")
                    if kind == "pair":
                        for i in range(2):
                            kb = 2 * a + i
                            # regions 0/1 are in different banks: each needs
                            # its own start=True (bank-granular zeroing)
                            nc.tensor.matmul(
                                sp[:, i, :],
                                kt_s[po:po + 64, mc, kb * 128:(kb + 1) * 128],
                                qt_t[po:po + 64, mc, :],
                                start=True, stop=True,
                                skip_group_check=True)
                        pt = ptp.tile([128, 2, 512], BF16, tag="pt",
                                      name=f"pt{n}_{h}_p{a}")
                        nc.scalar.activation(
                            pt[:, :, :], sp[:, :, :],
                            func=mybir.ActivationFunctionType.Exp, scale=ESC)
                        pt_of[2 * a] = (pt, 0)
                        pt_of[2 * a + 1] = (pt, 1)
                    else:
                        # diagonal pair d: blocks jj = 2d, 2d+1 share one
                        # 2-bank psum tile and ONE exp; unwritten psum cols
                        # are zeroed by start=True (exp(0*s)=1, never read)
                        d = a
                        col0x = 256 * d
                        for i in range(2):
                            kb = 4 * n + 2 * d + i
                            jj = 2 * d + i
                            col0s = min(jj * 128, 256)
                            nc.tensor.matmul(
                                sp[:, i, col0s:],
                                kt_s[po:po + 64, mc, kb * 128:(kb + 1) * 128],
                                qt_t[po:po + 64, mc, col0s:],
                                start=True, stop=True, skip_group_check=True)
                        pt = ptp.tile([128, 2, 512], BF16, tag="pt",
                                      name=f"pt{n}_{h}_dp{d}")
                        nc.scalar.activation(
                            pt[:, :, col0x:], sp[:, :, col0x:],
                            func=mybir.ActivationFunctionType.Exp, scale=ESC)
                        for i in range(2):
                            jj = 2 * d + i
                            # out-of-place mask: unmasked consumers (j > jj)
                            # read pt directly without waiting on DVE
                            ptm = ptp.tile([128, 128], BF16, tag="ptm",
                                           name=f"ptm{n}_{h}_{2 * d + i}",
                                           bufs=4)
                            nc.vector.tensor_mul(
                                ptm[:, :],
                                pt[:, i, jj * 128:(jj + 1) * 128], msk_s[:, :])
                            pt_of[4 * n + jj] = (pt, i)
                            ptm_of[4 * n + jj] = ptm

                def emit_av(u):
                    kind, a = u
                    if kind == "pair":
                        kbs = [2 * a, 2 * a + 1]
                    else:
                        kbs = [4 * n + 2 * a, 4 * n + 2 * a + 1]
                    for kb in kbs:
                        j0 = max(0, kb - 4 * n)
                        pt, reg = pt_of[kb]
                        for j in range(j0, 4):
                            if j == kb - 4 * n:
                                lhs = ptm_of[kb][:, :]
                            else:
                                lhs = pt[:, reg, j * 128:(j + 1) * 128]
                            # whole-bank zero on start: only first mm sets it
                            nc.tensor.matmul(
                                avps[:, j, :], lhs,
                                vaug_s[:, kb, h * 65:(h + 1) * 65],
                                start=(kb == 0 and j == 0),
                                stop=(kb == 4 * n + j),
                                skip_group_check=True)

                def finalize():
                    rc = rcp.tile([128, 4], F32, tag="rc", name=f"rc{n}_{h}")
                    nc.vector.reciprocal(rc[:, :], avps[:, :, 64])
                    for j in range(4):
                        nc.vector.tensor_scalar_mul(
                            ctxn_tiles[(n, j)][:, h * 64:(h + 1) * 64],
                            avps[:, j, 0:64], rc[:, j:j + 1])

                s_emit = [(lambda u: (lambda: emit_s(u)))(u) for u in sunits]
                av_emit = [(lambda u: (lambda: emit_av(u)))(u) for u in sunits]
                return s_emit, av_emit, finalize

            # ---- phase C unit: token tile t, output half n2 ----
            out_tiles = {}

            def c_unit(n, j, n2):
                t = 4 * n + j
                # C units share the psA ring (A units are gone or sparse
                # by the time C drains); psC's bank went to psAV=2 which
                # unserializes consecutive heads' AV accumulation
                pool = psA
                ptag = "pa"

                def emit():
                    if n2 == 0 and n < NSL - 1:
                        out_tiles[t] = outsp.tile([128, D], F32, tag="outs",
                                                  name=f"out{t}")
                    cps = pool.tile([128, 512], F32, tag=ptag,
                                    name=f"cps{t}_{n2}")
                    ctxT_t = ctxT_tiles[(n, j)]
                    for c in range(4):
                        nc.tensor.matmul(
                            cps[:, :], ctxT_t[:, c, :],
                            wo_s[:, c, n2 * 512:(n2 + 1) * 512],
                            start=(c == 0), stop=(c == 3))
                    if n == NSL - 1:
                        # tail: per-half copy + immediate DMA (faster outs
                        # ring recycle than full-row tiles)
                        oh = outsp.tile([128, 512], F32, tag="outs",
                                        name=f"outh{t}_{n2}")
                        nc.vector.tensor_copy(oh[:, :], cps[:, :])
                        nc.sync.dma_start(
                            outp[t * 128:(t + 1) * 128,
                                 n2 * 512:(n2 + 1) * 512], oh[:, :])
                    else:
                        nc.vector.tensor_copy(
                            out_tiles[t][:, n2 * 512:(n2 + 1) * 512],
                            cps[:, :])
                        if n2 == 1:
                            nc.sync.dma_start(
                                outp[t * 128:(t + 1) * 128, :],
                                out_tiles[t][:, :])
                return emit

            # ---- main schedule ----
            for u in a_units(0):
                u()

            a_queue = []          # (slice, unit) in slice order
            carry = None          # next slice's pre-planned head 0
            pending_c = []
            c_rate = dict(enumerate(CRATES))

            for n in range(NSL):
                if LOOKAHEAD:
                    if n == 0:
                        issue_xin(1)
                        issue_xin(2)
                        for u in a_units(1):
                            a_queue.append((1, u))
                        for u in a_units(2):
                            a_queue.append((2, u))
                    elif n == 1:
                        issue_xin(3)
                        for u in a_units(3):
                            a_queue.append((3, u))
                else:
                    if n < NSL - 1:
                        issue_xin(n + 1)
                        for u in a_units(n + 1):
                            a_queue.append((n + 1, u))
                for j in range(4):
                    ctxn_tiles[(n, j)] = ctxnp.tile(
                        [128, GW], BF16, tag="ctxn", name=f"ctxn{n}_{j}")
                fillq = []
                for _ in range(ARATE * HD):
                    if a_queue:
                        fillq.append(a_queue.pop(0)[1])
                for _ in range(c_rate.get(n, 2) * HD):
                    if pending_c:
                        fillq.append(pending_c.pop(0))
                if WEAVE:
                    prev_fin = None
                    for h in range(HD):
                        s_emit, av_emit, fin = plan_head(n, h)
                        ns_ = len(s_emit)
                        s_emit[0]()
                        if prev_fin is not None:
                            prev_fin()
                        s_emit[1]()
                        if fillq:
                            fillq.pop(0)()
                        for i in range(2, ns_):
                            s_emit[i]()
                            av_emit[i - 2]()
                            if fillq:
                                fillq.pop(0)()
                        av_emit[ns_ - 2]()
                        av_emit[ns_ - 1]()
                        prev_fin = fin
                    prev_fin()
                    while fillq:
                        fillq.pop(0)()
                else:
                    nfill = len(fillq)
                    for h in range(HD):
                        if h == 0 and carry is not None:
                            s_emit, av_emit, fin = carry
                            carry = None
                        else:
                            s_emit, av_emit, fin = plan_head(n, h)
                            s_emit[0]()
                            if len(s_emit) > 1:
                                s_emit[1]()
                        ns_ = len(s_emit)
                        for i in range(2, ns_):
                            s_emit[i]()
                            av_emit[i - 2]()
                        av_emit[ns_ - 2]()
                        av_emit[ns_ - 1]()
                        fin()
                        take = (nfill * (h + 1)) // HD - (nfill * h) // HD
                        for _ in range(take):
                            if fillq:
                                fillq.pop(0)()
                    # pre-emit next slice's h0 S prologue BEFORE the leftover
                    # A-drain (V halves): its Q/K units are already drained,
                    # so ACT stays fed across the slice boundary
                    if n < NSL - 1:
                        nxt = plan_head(n + 1, 0)
                        nxt[0][0]()
                        nxt[0][1]()
                        carry = nxt
                # B(n+1) needs all of A(n+1) done
                while a_queue and a_queue[0][0] <= n + 1:
                    a_queue.pop(0)[1]()
                if n < NSL - 1:
                    for j in range(4):
                        ct = ctxTp.tile([128, 4, 128], BF16, tag="ctxT",
                                        name=f"ctxT{n}_{j}")
                        nc.sync.dma_start_transpose(ct,
                                                    ctxn_tiles[(n, j)][:, :])
                        ctxT_tiles[(n, j)] = ct
                    for j in range(4):
                        for n2 in range(2):
                            pending_c.append(c_unit(n, j, n2))
                else:
                    # tail: PE transposes (PE is idle here) + DVE copy cut
                    # the ~2.8us-per-tile xbar DMA latency off the critical
                    # path; interleave each transpose with its C unit
                    for j in range(4):
                        tp = psA.tile([128, 4, 128], BF16, tag="pa",
                                      name=f"tp{j}")
                        cn_t = ctxn_tiles[(n, j)]
                        for c in range(4):
                            nc.tensor.transpose(tp[:, c, :],
                                                cn_t[:, c * 128:(c + 1) * 128],
                                                idn_s[:, :])
                        ct = ctxTp.tile([128, 4, 128], BF16, tag="ctxT",
                                        name=f"ctxT{n}_{j}")
                        nc.vector.tensor_copy(ct[:, :, :], tp[:, :, :])
                        ctxT_tiles[(n, j)] = ct
                        pending_c.append(c_unit(n, j, 0))
                        pending_c.append(c_unit(n, j, 1))
            while pending_c:
                pending_c.pop(0)()

            if dbg:
                nc.sync.dma_start(qt_dbg[:, :, :],
                                  qt_tiles[0][:, :, :].bitcast(F32))
                nc.sync.dma_start(kt_dbg[:, :, :], kt_s[:, :, :].bitcast(F32))
                nc.sync.dma_start(vg_dbg[:, :, :], vaug_s[:, :, :])
                for n in range(NSL):
                    for j in range(4):
                        nc.sync.dma_start(cn_dbg[:, 4 * n + j, :],
                                          ctxn_tiles[(n, j)][:, :])
                        nc.sync.dma_start(ct_dbg[:, 4 * n + j, :, :],
                                          ctxT_tiles[(n, j)][:, :, :])

    nc.compile()
    return nc


_NC = None
LAST_RESULTS = None


def kernel(**inputs):
    global _NC, LAST_RESULTS
    import os
    if _NC is None:
        _NC = _build_nc()

    f = lambda a: np.asarray(a, dtype=np.float32)
    q, k, v = f(inputs["q"]), f(inputs["k"]), f(inputs["v"])
    wq_w, wq_b = f(inputs["wq_w"]), f(inputs["wq_b"])
    wk_w, wk_b = f(inputs["wk_w"]), f(inputs["wk_b"])
    wv_w, wv_b = f(inputs["wv_w"]), f(inputs["wv_b"])
    wo_w, wo_b = f(inputs["wo_w"]), f(inputs["wo_b"])

    bf = ml_dtypes.bfloat16
    f8 = ml_dtypes.float8_e4m3

    def chunk_rows(a, inner):
        # [1024, X] -> [128, 8, X] with row r = c*128+p -> [p, c, :]
        return np.ascontiguousarray(
            a.reshape(NCH, 128, inner).transpose(1, 0, 2))

    msk = np.ascontiguousarray(
        (np.arange(128)[None, :] >= np.arange(128)[:, None])).astype(bf)

    gmaps = []
    for g in range(2):
        sl = slice(g * GW, (g + 1) * GW)
        wqT = chunk_rows(wq_w[sl].T, GW).astype(bf)
        wkT = chunk_rows(wk_w[sl].T, GW).astype(bf)
        wvT = np.zeros((D, AUGW), np.float32)
        vbias = np.zeros((AUGW,), np.float32)
        for h in range(HD):
            wvT[:, h * 65:h * 65 + 64] = wv_w[g * GW + h * 64:
                                              g * GW + (h + 1) * 64].T * WS
            vbias[h * 65:h * 65 + 64] = wv_b[g * GW + h * 64:
                                             g * GW + (h + 1) * 64] * WS
            vbias[h * 65 + 64] = WS
        woT = np.ascontiguousarray(
            wo_w[:, sl].T.reshape(4, 128, D).transpose(1, 0, 2)).astype(bf)
        bqT = np.ascontiguousarray(wq_b[sl].reshape(4, 128).T)
        bkT = np.ascontiguousarray(wk_b[sl].reshape(4, 128).T)
        gmaps.append(dict(wq=wqT, wk=wkT, wv=chunk_rows(wvT, AUGW).astype(bf),
                          wo=woT, bq=bqT, bk=bkT, vb=vbias, msk=msk,
                          idn=np.eye(128, dtype=np.float32).astype(bf)))

    bmaps = []
    for b in range(B):
        bmaps.append(dict(
            xq=chunk_rows(np.ascontiguousarray(q[b].T), L).astype(bf),
            xk=chunk_rows(np.ascontiguousarray(k[b].T), L).astype(bf),
            xv=chunk_rows(np.ascontiguousarray(v[b].T), L).astype(bf)))

    in_maps = [dict(**bmaps[c // 2], **gmaps[c % 2]) for c in range(8)]

    trace = bool(int(os.environ.get("KERNEL_TRACE", "0")))
    res = run_bass_kernel_spmd(_NC, in_maps, list(range(8)), trace=trace)
    LAST_RESULTS = res

    out = np.empty((B, L, D), np.float32)
    for b in range(B):
        out[b] = (res.results[2 * b]["outp"] + res.results[2 * b + 1]["outp"]
                  + wo_b[None, :])
    return out


# revision 55
# speedup vs baseline: 1.0103x; 1.0050x over previous
"""Causal MHA (B=4, L=2048, D=1024, H=16) on 8 NeuronCores.

Sharding: core c -> (batch b = c//2, head-group g = c%2). Data-parallel over
the 4 batches, tensor-parallel over heads (8 heads per core): wq/wk/wv
column-parallel, wo row-parallel. Each core returns a partial [L, D] output;
the host sums the two head-group partials per batch and adds wo_b.

Single fused streaming kernel, no DRAM round-trips:
  A(n):  Q/K/V projections in bf16 (x/w bf16, psum f32, Q/K kept f32r in
         SBUF for exact S logits). Emitted as ~0.9us units interleaved
         between B(n-1) heads to keep the PE busy while ACT runs exp.
  B(n):  per head: S.T[keys,q] = KT_h.T @ QT_h (f32r, causal-trimmed to
         N>=256), exp on ACT with full-history kb blocks PAIRED into
         [128,2,512] psum tiles (halves ACT per-instruction overhead) ->
         pt bf16; tri-mask on the diagonal tile (DVE); flipped AV:
         avps[q, 4t, 65] += pt_blk.T @ vaug (bf16, ones column = softmax
         denominator landing on the partition axis) -> per-partition
         reciprocal + tensor_scalar_mul normalize (no broadcasts).
  T(n):  ctxn [q,512] -> ctxT [d,q] via xbar DMA-transpose (off the PE);
         the LAST slice uses PE transposes + DVE copy instead (PE is idle
         at the tail and this cuts the xbar DMA latency off the critical
         path).
  C(n):  out[tok,1024] = sum_c ctxT[c].T @ wo[c] (bf16), drained 2-per-head
         into the ACT-bound later slices on the psA psum ring; the tail
         writes per-512-column halves for faster outs-ring recycle.
PSUM budget (8 banks): S pairs 2x2 + AV accum 2 (unserializes consecutive
heads) + shared A-proj/C/transpose ring 2.
"""

import numpy as np
import ml_dtypes

import concourse.bacc as bacc
import concourse.bass as bass
import concourse.mybir as mybir
import concourse.tile as tile
from concourse.bass_utils import run_bass_kernel_spmd

F32 = mybir.dt.float32
F32R = mybir.dt.float32r
BF16 = mybir.dt.bfloat16
F8 = mybir.dt.float8e4
DR = mybir.MatmulPerfMode.DoubleRow

B, L, D, H, DK = 4, 2048, 1024, 16, 64
HD = 8             # heads per core
GW = 512           # head-group width (8 heads * 64)
AUGW = HD * (DK + 1)   # 520: per head 64 dims + ones col (ones LAST per head)
NCH = D // 128     # 8 contraction chunks
NSL = 4            # token slices of 512
NTT = L // 128     # 16 token tiles
WS = 1.0           # V-path pre-scale (cancels in softmax ratio; 1 for bf16)
ESC = 0.125        # exp scale: 1/sqrt(DK)
EBI = -2.0         # exp bias: shift-invariant headroom so exp fits fp8e4m3


def _build_nc(dbg=False):
    import os
    LOOKAHEAD = bool(int(os.environ.get("KCFG_LOOKAHEAD", "0")))
    ARATE = int(os.environ.get("KCFG_ARATE", "2"))
    CRATES = [int(x) for x in os.environ.get("KCFG_CRATE", "2,2,2,2").split(",")]
    INHEAD = bool(int(os.environ.get("KCFG_INHEAD", "0")))
    BIASACT = bool(int(os.environ.get("KCFG_BIASACT", "0")))
    WEAVE = bool(int(os.environ.get("KCFG_WEAVE", "0")))
    nc = bacc.Bacc("TRN2", target_bir_lowering=False, debug=False, num_devices=8)

    xq = nc.dram_tensor("xq", [128, NCH, L], BF16, kind="ExternalInput").ap()
    xk = nc.dram_tensor("xk", [128, NCH, L], BF16, kind="ExternalInput").ap()
    xv = nc.dram_tensor("xv", [128, NCH, L], BF16, kind="ExternalInput").ap()
    wq = nc.dram_tensor("wq", [128, NCH, GW], BF16, kind="ExternalInput").ap()
    wk = nc.dram_tensor("wk", [128, NCH, GW], BF16, kind="ExternalInput").ap()
    wv = nc.dram_tensor("wv", [128, NCH, AUGW], BF16, kind="ExternalInput").ap()
    wo = nc.dram_tensor("wo", [128, 4, D], BF16, kind="ExternalInput").ap()
    bq = nc.dram_tensor("bq", [128, 4], F32, kind="ExternalInput").ap()
    bk = nc.dram_tensor("bk", [128, 4], F32, kind="ExternalInput").ap()
    vb = nc.dram_tensor("vb", [AUGW], F32, kind="ExternalInput").ap()
    msk = nc.dram_tensor("msk", [128, 128], BF16, kind="ExternalInput").ap()
    idn = nc.dram_tensor("idn", [128, 128], BF16, kind="ExternalInput").ap()
    outp = nc.dram_tensor("outp", [L, D], F32, kind="ExternalOutput").ap()
    if dbg:
        qt_dbg = nc.dram_tensor("qt_dbg", [128, 4, GW], F32,
                                kind="ExternalOutput").ap()
        kt_dbg = nc.dram_tensor("kt_dbg", [128, 4, L], F32,
                                kind="ExternalOutput").ap()
        vg_dbg = nc.dram_tensor("vg_dbg", [128, NTT, AUGW], BF16,
                                kind="ExternalOutput").ap()
        cn_dbg = nc.dram_tensor("cn_dbg", [128, NTT, GW], BF16,
                                kind="ExternalOutput").ap()
        ct_dbg = nc.dram_tensor("ct_dbg", [128, NTT, 4, 128], BF16,
                                kind="ExternalOutput").ap()

    with tile.TileContext(nc) as tc:
        with (
            tc.tile_pool(name="persist", bufs=1) as persist,
            tc.tile_pool(name="qtp", bufs=(4 if dbg else 3)) as qtp,
            tc.tile_pool(name="xqk", bufs=(3 if dbg else 4)) as xqkp,
            tc.tile_pool(name="xvp", bufs=2) as xvp,
            tc.tile_pool(name="ptp", bufs=(3 if dbg else 5)) as ptp,
            tc.tile_pool(name="ctxn", bufs=(16 if dbg else 6)) as ctxnp,
            tc.tile_pool(name="ctxT", bufs=16) as ctxTp,
            tc.tile_pool(name="rcp", bufs=4) as rcp,
            tc.tile_pool(name="outs", bufs=(2 if dbg else 4)) as outsp,
            tc.tile_pool(name="psS", bufs=2, space="PSUM") as psS,
            tc.tile_pool(name="psAV", bufs=2, space="PSUM") as psAV,
            tc.tile_pool(name="psA", bufs=2, space="PSUM") as psA,
        ):
            # ---- persistent SBUF ----
            wq_s = persist.tile([128, NCH, GW], BF16, tag="wq")
            wk_s = persist.tile([128, NCH, GW], BF16, tag="wk")
            wv_s = persist.tile([128, NCH, AUGW], BF16, tag="wv")
            wo_s = persist.tile([128, 4, D], BF16, tag="wo")
            kt_s = persist.tile([128, 4, L], F32R, tag="kt")
            vaug_s = persist.tile([128, NTT, AUGW], BF16, tag="vaug")
            bq_s = persist.tile([128, 4], F32, tag="bq")
            bk_s = persist.tile([128, 4], F32, tag="bk")
            vb_s = persist.tile([128, AUGW], BF16, tag="vb")
            msk_s = persist.tile([128, 128], BF16, tag="msk")
            idn_s = persist.tile([128, 128], BF16, tag="idn")

            # weight/const loads; order = DMA engine order (startup latency)
            xq_tiles = {}
            xk_tiles = {}
            xv_tiles = {}

            def issue_xin(n):
                c0, c1 = n * 512, (n + 1) * 512
                t = xqkp.tile([128, NCH, 512], BF16, tag="xqk", name=f"xq{n}")
                nc.sync.dma_start(t[:, :, :], xq[:, :, c0:c1])
                xq_tiles[n] = t
                t = xqkp.tile([128, NCH, 512], BF16, tag="xqk", name=f"xk{n}")
                nc.sync.dma_start(t[:, :, :], xk[:, :, c0:c1])
                xk_tiles[n] = t
                t = xvp.tile([128, NCH, 512], BF16, tag="xv", name=f"xv{n}")
                nc.sync.dma_start(t[:, :, :], xv[:, :, c0:c1])
                xv_tiles[n] = t

            # startup order matches phase-A(0) consumption: alternate
            # Q/K weight+input quarters so the first 4 QK units stream in
            tq = xqkp.tile([128, NCH, 512], BF16, tag="xqk", name="xq0")
            tk = xqkp.tile([128, NCH, 512], BF16, tag="xqk", name="xk0")
            nc.sync.dma_start(wq_s[:, :, 0:256], wq[:, :, 0:256])
            nc.sync.dma_start(tq[:, :, 0:256], xq[:, :, 0:256])
            nc.sync.dma_start(wk_s[:, :, 0:256], wk[:, :, 0:256])
            nc.sync.dma_start(tk[:, :, 0:256], xk[:, :, 0:256])
            nc.sync.dma_start(bq_s[:, :], bq[:, :])
            nc.sync.dma_start(bk_s[:, :], bk[:, :])
            nc.sync.dma_start(wq_s[:, :, 256:512], wq[:, :, 256:512])
            nc.sync.dma_start(tq[:, :, 256:512], xq[:, :, 256:512])
            nc.sync.dma_start(wk_s[:, :, 256:512], wk[:, :, 256:512])
            nc.sync.dma_start(tk[:, :, 256:512], xk[:, :, 256:512])
            xq_tiles[0] = tq
            xk_tiles[0] = tk
            nc.sync.dma_start(wv_s[:, :, :], wv[:, :, :])
            t0 = xvp.tile([128, NCH, 512], BF16, tag="xv", name="xv0")
            nc.sync.dma_start(t0[:, :, :], xv[:, :, 0:512])
            xv_tiles[0] = t0
            vb_bcast = bass.AP(tensor=vb.tensor, offset=vb.offset,
                               ap=[[0, 128], [1, AUGW]])
            nc.gpsimd.dma_start(vb_s[:, :], vb_bcast)
            nc.sync.dma_start(msk_s[:, :], msk[:, :])
            nc.sync.dma_start(idn_s[:, :], idn[:, :])
            nc.sync.dma_start(wo_s[:, :, :], wo[:, :, :])

            qt_tiles = {}

            # ---- phase A unit generator: fp8 DoubleRow projections ----
            def a_units(n):
                qt_t = qtp.tile([128, 4, GW], F32R, tag="qt", name=f"qt{n}")
                qt_tiles[n] = qt_t

                def qk_unit(hf, g, x_of, w_s, b_s, is_q):
                    # split into two ~0.9us halves (one mi each) so fillers
                    # can weave between S/exp steps without starving ACT
                    ps_box = []

                    def half(mi):
                        def emit():
                            x_t = x_of[n]
                            if mi == 0:
                                ps_box.append(psA.tile(
                                    [128, 2, 256], F32, tag="pa",
                                    name=f"pa{n}_{hf}_{g}"))
                            ps = ps_box[0]
                            for c in range(NCH):
                                # start=True zeroes the whole psum bank:
                                # only the first write into the tile sets it
                                nc.tensor.matmul(
                                    ps[:, mi, :],
                                    w_s[:, c, (2 * g + mi) * 128:
                                        (2 * g + mi + 1) * 128],
                                    x_t[:, c, hf * 256:hf * 256 + 256],
                                    start=(c == 0 and mi == 0),
                                    stop=(c == NCH - 1),
                                    skip_group_check=True)
                            m = 2 * g + mi
                            if is_q:
                                dst = qt_t[:, m, hf * 256:hf * 256 + 256]
                            else:
                                dst = kt_s[:, m, n * 512 + hf * 256:
                                           n * 512 + hf * 256 + 256]
                            if BIASACT:
                                nc.scalar.activation(
                                    dst, ps[:, mi, :],
                                    func=mybir.ActivationFunctionType.Identity,
                                    bias=b_s[:, m:m + 1])
                            else:
                                nc.vector.tensor_scalar_add(
                                    dst, ps[:, mi, :], b_s[:, m:m + 1])
                        return emit
                    return [half(0), half(1)]

                def v_unit(tt, vhf):
                    def emit():
                        ps = psA.tile([128, 260], F32, tag="pa",
                                      name=f"pv{n}_{tt}_{vhf}")
                        xv_t = xv_tiles[n]
                        for c in range(NCH):
                            nc.tensor.matmul(
                                ps[:, :],
                                xv_t[:, c, tt * 128:(tt + 1) * 128],
                                wv_s[:, c, vhf * 260:(vhf + 1) * 260],
                                start=(c == 0), stop=(c == NCH - 1))
                        nc.vector.tensor_add(
                            vaug_s[:, n * 4 + tt, vhf * 260:(vhf + 1) * 260],
                            ps[:, :], vb_s[:, vhf * 260:(vhf + 1) * 260])
                    return emit

                units = []
                for hf in range(2):
                    for g in range(2):
                        units.extend(qk_unit(hf, g, xq_tiles, wq_s, bq_s, True))
                        units.extend(qk_unit(hf, g, xk_tiles, wk_s, bk_s,
                                             False))
                for hf in range(2):
                    for tt in (2 * hf, 2 * hf + 1):
                        for vhf in range(2):
                            units.append(v_unit(tt, vhf))
                return units

            ctxn_tiles = {}
            ctxT_tiles = {}

            # ---- phase B: one head of slice n ----
            # returns (s_emitters, av_emitters, finalize) so the slice loop
            # can weave the next head's first S blocks before this head's
            # tail, keeping ACT fed across head boundaries
            def plan_head(n, h):
                po = (h % 2) * 64
                mc = h // 2
                qt_t = qt_tiles[n]
                nkb = 4 * n + 4
                avps = psAV.tile([128, 4, DK + 1], F32, tag="av",
                                 name=f"av{n}_{h}")
                # S/exp units: full-history kb pairs, then 2 diag pairs
                pt_of = {}   # kb -> (tile, region)
                ptm_of = {}  # kb -> masked diag tile
                sunits = [("pair", p) for p in range(2 * n)]
                sunits += [("dpair", 0), ("dpair", 1)]

                def emit_s(u):
                    kind, a = u
                    sp = psS.tile([128, 2, 512], F32, tag="sp",
                                  name=f"sp{n}_{h}_{kind}{a}")
                    if kind == "pair":
                        for i in range(2):
                            kb = 2 * a + i
                            # regions 0/1 are in different banks: each needs
                            # its own start=True (bank-granular zeroing)
                            nc.tensor.matmul(
                                sp[:, i, :],
                                kt_s[po:po + 64, mc, kb * 128:(kb + 1) * 128],
                                qt_t[po:po + 64, mc, :],
                                start=True, stop=True,
                                skip_group_check=True)
                        pt = ptp.tile([128, 2, 512], BF16, tag="pt",
                                      name=f"pt{n}_{h}_p{a}")
                        nc.scalar.activation(
                            pt[:, :, :], sp[:, :, :],
                            func=mybir.ActivationFunctionType.Exp, scale=ESC)
                        pt_of[2 * a] = (pt, 0)
                        pt_of[2 * a + 1] = (pt, 1)
                    else:
                        # diagonal pair d: blocks jj = 2d, 2d+1 share one
                        # 2-bank psum tile and ONE exp; unwritten psum cols
                        # are zeroed by start=True (exp(0*s)=1, never read)
                        d = a
                        col0x = 256 * d
                        for i in range(2):
                            kb = 4 * n + 2 * d + i
                            jj = 2 * d + i
                            col0s = min(jj * 128, 256)
                            nc.tensor.matmul(
                                sp[:, i, col0s:],
                                kt_s[po:po + 64, mc, kb * 128:(kb + 1) * 128],
                                qt_t[po:po + 64, mc, col0s:],
                                start=True, stop=True, skip_group_check=True)
                        pt = ptp.tile([128, 2, 512], BF16, tag="pt",
                                      name=f"pt{n}_{h}_dp{d}")
                        nc.scalar.activation(
                            pt[:, :, col0x:], sp[:, :, col0x:],
                            func=mybir.ActivationFunctionType.Exp, scale=ESC)
                        for i in range(2):
                            jj = 2 * d + i
                            # out-of-place mask: unmasked consumers (j > jj)
                            # read pt directly without waiting on DVE
                            ptm = ptp.tile([128, 128], BF16, tag="ptm",
                                           name=f"ptm{n}_{h}_{2 * d + i}",
                                           bufs=4)
                            nc.vector.tensor_mul(
                                ptm[:, :],
                                pt[:, i, jj * 128:(jj + 1) * 128], msk_s[:, :])
                            pt_of[4 * n + jj] = (pt, i)
                            ptm_of[4 * n + jj] = ptm

                def emit_av(u):
                    kind, a = u
                    if kind == "pair":
                        kbs = [2 * a, 2 * a + 1]
                    else:
                        kbs = [4 * n + 2 * a, 4 * n + 2 * a + 1]
                    for kb in kbs:
                        j0 = max(0, kb - 4 * n)
                        pt, reg = pt_of[kb]
                        for j in range(j0, 4):
                            if j == kb - 4 * n:
                                lhs = ptm_of[kb][:, :]
                            else:
                                lhs = pt[:, reg, j * 128:(j + 1) * 128]
                            # whole-bank zero on start: only first mm sets it
                            nc.tensor.matmul(
                                avps[:, j, :], lhs,
                                vaug_s[:, kb, h * 65:(h + 1) * 65],
                                start=(kb == 0 and j == 0),
                                stop=(kb == 4 * n + j),
                                skip_group_check=True)

                def finalize():
                    rc = rcp.tile([128, 4], F32, tag="rc", name=f"rc{n}_{h}")
                    nc.vector.reciprocal(rc[:, :], avps[:, :, 64])
                    for j in range(4):
                        nc.vector.tensor_scalar_mul(
                            ctxn_tiles[(n, j)][:, h * 64:(h + 1) * 64],
                            avps[:, j, 0:64], rc[:, j:j + 1])

                s_emit = [(lambda u: (lambda: emit_s(u)))(u) for u in sunits]
                av_emit = [(lambda u: (lambda: emit_av(u)))(u) for u in sunits]
                return s_emit, av_emit, finalize

            # ---- phase C unit: token tile t, output half n2 ----
            out_tiles = {}

            def c_unit(n, j, n2):
                t = 4 * n + j
                # C units share the psA ring (A units are gone or sparse
                # by the time C drains); psC's bank went to psAV=2 which
                # unserializes consecutive heads' AV accumulation
                pool = psA
                ptag = "pa"

                def emit():
                    if n2 == 0 and n < NSL - 1:
                        out_tiles[t] = outsp.tile([128, D], F32, tag="outs",
                                                  name=f"out{t}")
                    cps = pool.tile([128, 512], F32, tag=ptag,
                                    name=f"cps{t}_{n2}")
                    ctxT_t = ctxT_tiles[(n, j)]
                    for c in range(4):
                        nc.tensor.matmul(
                            cps[:, :], ctxT_t[:, c, :],
                            wo_s[:, c, n2 * 512:(n2 + 1) * 512],
                            start=(c == 0), stop=(c == 3))
                    if n == NSL - 1:
                        # tail: per-half copy + immediate DMA (faster outs
                        # ring recycle than full-row tiles)
                        oh = outsp.tile([128, 512], F32, tag="outs",
                                        name=f"outh{t}_{n2}")
                        nc.vector.tensor_copy(oh[:, :], cps[:, :])
                        nc.sync.dma_start(
                            outp[t * 128:(t + 1) * 128,
                                 n2 * 512:(n2 + 1) * 512], oh[:, :])
                    else:
                        nc.vector.tensor_copy(
                            out_tiles[t][:, n2 * 512:(n2 + 1) * 512],
                            cps[:, :])
                        if n2 == 1:
                            nc.sync.dma_start(
                                outp[t * 128:(t + 1) * 128, :],
                                out_tiles[t][:, :])
                return emit

            # ---- main schedule ----
            for u in a_units(0):
                u()

            a_queue = []          # (slice, unit) in slice order
            carry = []            # next slice's pre-planned heads
            pending_c = []
            c_rate = dict(enumerate(CRATES))

            for n in range(NSL):
                if LOOKAHEAD:
                    if n == 0:
                        issue_xin(1)
                        issue_xin(2)
                        for u in a_units(1):
                            a_queue.append((1, u))
                        for u in a_units(2):
                            a_queue.append((2, u))
                    elif n == 1:
                        issue_xin(3)
                        for u in a_units(3):
                            a_queue.append((3, u))
                else:
                    if n < NSL - 1:
                        issue_xin(n + 1)
                        for u in a_units(n + 1):
                            a_queue.append((n + 1, u))
                for j in range(4):
                    ctxn_tiles[(n, j)] = ctxnp.tile(
                        [128, GW], BF16, tag="ctxn", name=f"ctxn{n}_{j}")
                fillq = []
                for _ in range(ARATE * HD):
                    if a_queue:
                        fillq.append(a_queue.pop(0)[1])
                for _ in range(c_rate.get(n, 2) * HD):
                    if pending_c:
                        fillq.append(pending_c.pop(0))
                if WEAVE:
                    prev_fin = None
                    for h in range(HD):
                        s_emit, av_emit, fin = plan_head(n, h)
                        ns_ = len(s_emit)
                        s_emit[0]()
                        if prev_fin is not None:
                            prev_fin()
                        s_emit[1]()
                        if fillq:
                            fillq.pop(0)()
                        for i in range(2, ns_):
                            s_emit[i]()
                            av_emit[i - 2]()
                            if fillq:
                                fillq.pop(0)()
                        av_emit[ns_ - 2]()
                        av_emit[ns_ - 1]()
                        prev_fin = fin
                    prev_fin()
                    while fillq:
                        fillq.pop(0)()
                else:
                    nfill = len(fillq)
                    for h in range(HD):
                        if h < len(carry):
                            s_emit, av_emit, fin = carry[h]
                        else:
                            s_emit, av_emit, fin = plan_head(n, h)
                            s_emit[0]()
                            if len(s_emit) > 1:
                                s_emit[1]()
                        ns_ = len(s_emit)
                        for i in range(2, ns_):
                            s_emit[i]()
                            av_emit[i - 2]()
                        av_emit[ns_ - 2]()
                        av_emit[ns_ - 1]()
                        fin()
                        take = (nfill * (h + 1)) // HD - (nfill * h) // HD
                        for _ in range(take):
                            if fillq:
                                fillq.pop(0)()
                    # pre-emit next slice's h0/h1 S prologues BEFORE the
                    # leftover A-drain (V halves): they only need the Q/K
                    # units already drained, so ACT stays fed across the
                    # slice boundary
                    carry = []
                    if n < NSL - 1:
                        for hh in range(2):
                            nxt = plan_head(n + 1, hh)
                            nxt[0][0]()
                            nxt[0][1]()
                            carry.append(nxt)
                # B(n+1) needs all of A(n+1) done
                while a_queue and a_queue[0][0] <= n + 1:
                    a_queue.pop(0)[1]()
                if n < NSL - 1:
                    for j in range(4):
                        ct = ctxTp.tile([128, 4, 128], BF16, tag="ctxT",
                                        name=f"ctxT{n}_{j}")
                        nc.sync.dma_start_transpose(ct,
                                                    ctxn_tiles[(n, j)][:, :])
                        ctxT_tiles[(n, j)] = ct
                    for j in range(4):
                        for n2 in range(2):
                            pending_c.append(c_unit(n, j, n2))
                else:
                    # tail: PE transposes (PE is idle here) + DVE copy cut
                    # the ~2.8us-per-tile xbar DMA latency off the critical
                    # path; interleave each transpose with its C unit
                    for j in range(4):
                        tp = psA.tile([128, 4, 128], BF16, tag="pa",
                                      name=f"tp{j}")
                        cn_t = ctxn_tiles[(n, j)]
                        for c in range(4):
                            nc.tensor.transpose(tp[:, c, :],
                                                cn_t[:, c * 128:(c + 1) * 128],
                                                idn_s[:, :])
                        ct = ctxTp.tile([128, 4, 128], BF16, tag="ctxT",
                                        name=f"ctxT{n}_{j}")
                        nc.vector.tensor_copy(ct[:, :, :], tp[:, :, :])
                        ctxT_tiles[(n, j)] = ct
                        pending_c.append(c_unit(n, j, 0))
                        pending_c.append(c_unit(n, j, 1))
            while pending_c:
                pending_c.pop(0)()

            if dbg:
                nc.sync.dma_start(qt_dbg[:, :, :],
                                  qt_tiles[0][:, :, :].bitcast(F32))
                nc.sync.dma_start(kt_dbg[:, :, :], kt_s[:, :, :].bitcast(F32))
                nc.sync.dma_start(vg_dbg[:, :, :], vaug_s[:, :, :])
                for n in range(NSL):
                    for j in range(4):
                        nc.sync.dma_start(cn_dbg[:, 4 * n + j, :],
                                          ctxn_tiles[(n, j)][:, :])
                        nc.sync.dma_start(ct_dbg[:, 4 * n + j, :, :],
                                          ctxT_tiles[(n, j)][:, :, :])

    nc.compile()
    return nc


_NC = None
LAST_RESULTS = None


def kernel(**inputs):
    global _NC, LAST_RESULTS
    import os
    if _NC is None:
        _NC = _build_nc()

    f = lambda a: np.asarray(a, dtype=np.float32)
    q, k, v = f(inputs["q"]), f(inputs["k"]), f(inputs["v"])
    wq_w, wq_b = f(inputs["wq_w"]), f(inputs["wq_b"])
    wk_w, wk_b = f(inputs["wk_w"]), f(inputs["wk_b"])
    wv_w, wv_b = f(inputs["wv_w"]), f(inputs["wv_b"])
    wo_w, wo_b = f(inputs["wo_w"]), f(inputs["wo_b"])

    bf = ml_dtypes.bfloat16
    f8 = ml_dtypes.float8_e4m3

    def chunk_rows(a, inner):
        # [1024, X] -> [128, 8, X] with row r = c*128+p -> [p, c, :]
        return np.ascontiguousarray(
            a.reshape(NCH, 128, inner).transpose(1, 0, 2))

    msk = np.ascontiguousarray(
        (np.arange(128)[None, :] >= np.arange(128)[:, None])).astype(bf)

    gmaps = []
    for g in range(2):
        sl = slice(g * GW, (g + 1) * GW)
        wqT = chunk_rows(wq_w[sl].T, GW).astype(bf)
        wkT = chunk_rows(wk_w[sl].T, GW).astype(bf)
        wvT = np.zeros((D, AUGW), np.float32)
        vbias = np.zeros((AUGW,), np.float32)
        for h in range(HD):
            wvT[:, h * 65:h * 65 + 64] = wv_w[g * GW + h * 64:
                                              g * GW + (h + 1) * 64].T * WS
            vbias[h * 65:h * 65 + 64] = wv_b[g * GW + h * 64:
                                             g * GW + (h + 1) * 64] * WS
            vbias[h * 65 + 64] = WS
        woT = np.ascontiguousarray(
            wo_w[:, sl].T.reshape(4, 128, D).transpose(1, 0, 2)).astype(bf)
        bqT = np.ascontiguousarray(wq_b[sl].reshape(4, 128).T)
        bkT = np.ascontiguousarray(wk_b[sl].reshape(4, 128).T)
        gmaps.append(dict(wq=wqT, wk=wkT, wv=chunk_rows(wvT, AUGW).astype(bf),
                          wo=woT, bq=bqT, bk=bkT, vb=vbias, msk=msk,
                          idn=np.eye(128, dtype=np.float32).astype(bf)))

    bmaps = []
    for b in range(B):
        bmaps.append(dict(
            xq=chunk_rows(np.ascontiguousarray(q[b].T), L).astype(bf),
            xk=chunk_rows(np.ascontiguousarray(k[b].T), L).astype(bf),
            xv=chunk_rows(np.ascontiguousarray(v[b].T), L).astype(bf)))

    in_maps = [dict(**bmaps[c // 2], **gmaps[c % 2]) for c in range(8)]

    trace = bool(int(os.environ.get("KERNEL_TRACE", "0")))
    res = run_bass_kernel_spmd(_NC, in_maps, list(range(8)), trace=trace)
    LAST_RESULTS = res

    out = np.empty((B, L, D), np.float32)
    for b in range(B):
        out[b] = (res.results[2 * b]["outp"] + res.results[2 * b + 1]["outp"]
                  + wo_b[None, :])
    return out


# revision 62
# speedup vs baseline: 1.0119x; 1.0016x over previous
"""Causal MHA (B=4, L=2048, D=1024, H=16) on 8 NeuronCores.

Sharding: core c -> (batch b = c//2, head-group g = c%2). Data-parallel over
the 4 batches, tensor-parallel over heads (8 heads per core): wq/wk/wv
column-parallel, wo row-parallel. Each core returns a partial [L, D] output;
the host sums the two head-group partials per batch and adds wo_b.

Single fused streaming kernel, no DRAM round-trips:
  A(n):  Q/K/V projections in bf16 (x/w bf16, psum f32, Q/K kept f32r in
         SBUF for exact S logits). Emitted as ~0.9us units interleaved
         between B(n-1) heads to keep the PE busy while ACT runs exp.
  B(n):  per head: S.T[keys,q] = KT_h.T @ QT_h (f32r, causal-trimmed to
         N>=256), exp on ACT with full-history kb blocks PAIRED into
         [128,2,512] psum tiles (halves ACT per-instruction overhead) ->
         pt bf16; tri-mask on the diagonal tile (DVE); flipped AV:
         avps[q, 4t, 65] += pt_blk.T @ vaug (bf16, ones column = softmax
         denominator landing on the partition axis) -> per-partition
         reciprocal + tensor_scalar_mul normalize (no broadcasts).
  T(n):  ctxn [q,512] -> ctxT [d,q] via xbar DMA-transpose (off the PE);
         the LAST slice uses PE transposes + DVE copy instead (PE is idle
         at the tail and this cuts the xbar DMA latency off the critical
         path).
  C(n):  out[tok,1024] = sum_c ctxT[c].T @ wo[c] (bf16), drained 2-per-head
         into the ACT-bound later slices on the psA psum ring; the tail
         writes per-512-column halves for faster outs-ring recycle.
PSUM budget (8 banks): S pairs 2x2 + AV accum 2 (unserializes consecutive
heads) + shared A-proj/C/transpose ring 2.
"""

import numpy as np
import ml_dtypes

import concourse.bacc as bacc
import concourse.bass as bass
import concourse.mybir as mybir
import concourse.tile as tile
from concourse.bass_utils import run_bass_kernel_spmd

F32 = mybir.dt.float32
F32R = mybir.dt.float32r
BF16 = mybir.dt.bfloat16
F8 = mybir.dt.float8e4
DR = mybir.MatmulPerfMode.DoubleRow

B, L, D, H, DK = 4, 2048, 1024, 16, 64
HD = 8             # heads per core
GW = 512           # head-group width (8 heads * 64)
AUGW = HD * (DK + 1)   # 520: per head 64 dims + ones col (ones LAST per head)
NCH = D // 128     # 8 contraction chunks
NSL = 4            # token slices of 512
NTT = L // 128     # 16 token tiles
WS = 1.0           # V-path pre-scale (cancels in softmax ratio; 1 for bf16)
ESC = 0.125        # exp scale: 1/sqrt(DK)
EBI = -2.0         # exp bias: shift-invariant headroom so exp fits fp8e4m3


def _build_nc(dbg=False):
    import os
    LOOKAHEAD = bool(int(os.environ.get("KCFG_LOOKAHEAD", "0")))
    ARATE = int(os.environ.get("KCFG_ARATE", "2"))
    CRATES = [int(x) for x in os.environ.get("KCFG_CRATE", "2,2,2,2").split(",")]
    INHEAD = bool(int(os.environ.get("KCFG_INHEAD", "0")))
    BIASACT = bool(int(os.environ.get("KCFG_BIASACT", "0")))
    WEAVE = bool(int(os.environ.get("KCFG_WEAVE", "0")))
    nc = bacc.Bacc("TRN2", target_bir_lowering=False, debug=False, num_devices=8)

    xq = nc.dram_tensor("xq", [128, NCH, L], BF16, kind="ExternalInput").ap()
    xk = nc.dram_tensor("xk", [128, NCH, L], BF16, kind="ExternalInput").ap()
    xv = nc.dram_tensor("xv", [128, NCH, L], BF16, kind="ExternalInput").ap()
    wq = nc.dram_tensor("wq", [128, NCH, GW], BF16, kind="ExternalInput").ap()
    wk = nc.dram_tensor("wk", [128, NCH, GW], BF16, kind="ExternalInput").ap()
    wv = nc.dram_tensor("wv", [128, NCH, AUGW], BF16, kind="ExternalInput").ap()
    wo = nc.dram_tensor("wo", [128, 4, D], BF16, kind="ExternalInput").ap()
    bq = nc.dram_tensor("bq", [128, 4], F32, kind="ExternalInput").ap()
    bk = nc.dram_tensor("bk", [128, 4], F32, kind="ExternalInput").ap()
    vb = nc.dram_tensor("vb", [AUGW], F32, kind="ExternalInput").ap()
    msk = nc.dram_tensor("msk", [128, 128], BF16, kind="ExternalInput").ap()
    idn = nc.dram_tensor("idn", [128, 128], BF16, kind="ExternalInput").ap()
    outp = nc.dram_tensor("outp", [L, D], F32, kind="ExternalOutput").ap()
    if dbg:
        qt_dbg = nc.dram_tensor("qt_dbg", [128, 4, GW], F32,
                                kind="ExternalOutput").ap()
        kt_dbg = nc.dram_tensor("kt_dbg", [128, 4, L], F32,
                                kind="ExternalOutput").ap()
        vg_dbg = nc.dram_tensor("vg_dbg", [128, NTT, AUGW], BF16,
                                kind="ExternalOutput").ap()
        cn_dbg = nc.dram_tensor("cn_dbg", [128, NTT, GW], BF16,
                                kind="ExternalOutput").ap()
        ct_dbg = nc.dram_tensor("ct_dbg", [128, NTT, 4, 128], BF16,
                                kind="ExternalOutput").ap()

    with tile.TileContext(nc) as tc:
        with (
            tc.tile_pool(name="persist", bufs=1) as persist,
            tc.tile_pool(name="qtp", bufs=(4 if dbg else 3)) as qtp,
            tc.tile_pool(name="xqk", bufs=(3 if dbg else 4)) as xqkp,
            tc.tile_pool(name="xvp", bufs=2) as xvp,
            tc.tile_pool(name="ptp", bufs=(3 if dbg else 5)) as ptp,
            tc.tile_pool(name="ctxn", bufs=(16 if dbg else 6)) as ctxnp,
            tc.tile_pool(name="ctxT", bufs=16) as ctxTp,
            tc.tile_pool(name="rcp", bufs=4) as rcp,
            tc.tile_pool(name="outs", bufs=(2 if dbg else 4)) as outsp,
            tc.tile_pool(name="psS", bufs=2, space="PSUM") as psS,
            tc.tile_pool(name="psAV", bufs=2, space="PSUM") as psAV,
            tc.tile_pool(name="psA", bufs=2, space="PSUM") as psA,
        ):
            # ---- persistent SBUF ----
            wq_s = persist.tile([128, NCH, GW], BF16, tag="wq")
            wk_s = persist.tile([128, NCH, GW], BF16, tag="wk")
            wv_s = persist.tile([128, NCH, AUGW], BF16, tag="wv")
            wo_s = persist.tile([128, 4, D], BF16, tag="wo")
            kt_s = persist.tile([128, 4, L], F32R, tag="kt")
            vaug_s = persist.tile([128, NTT, AUGW], BF16, tag="vaug")
            bq_s = persist.tile([128, 4], F32, tag="bq")
            bk_s = persist.tile([128, 4], F32, tag="bk")
            vb_s = persist.tile([128, AUGW], BF16, tag="vb")
            msk_s = persist.tile([128, 128], BF16, tag="msk")
            idn_s = persist.tile([128, 128], BF16, tag="idn")

            # weight/const loads; order = DMA engine order (startup latency)
            xq_tiles = {}
            xk_tiles = {}
            xv_tiles = {}

            def issue_xin(n):
                c0, c1 = n * 512, (n + 1) * 512
                t = xqkp.tile([128, NCH, 512], BF16, tag="xqk", name=f"xq{n}")
                nc.sync.dma_start(t[:, :, :], xq[:, :, c0:c1])
                xq_tiles[n] = t
                t = xqkp.tile([128, NCH, 512], BF16, tag="xqk", name=f"xk{n}")
                nc.sync.dma_start(t[:, :, :], xk[:, :, c0:c1])
                xk_tiles[n] = t
                t = xvp.tile([128, NCH, 512], BF16, tag="xv", name=f"xv{n}")
                nc.sync.dma_start(t[:, :, :], xv[:, :, c0:c1])
                xv_tiles[n] = t

            # startup order matches phase-A(0) consumption: alternate
            # Q/K weight+input quarters so the first 4 QK units stream in
            tq = xqkp.tile([128, NCH, 512], BF16, tag="xqk", name="xq0")
            tk = xqkp.tile([128, NCH, 512], BF16, tag="xqk", name="xk0")
            nc.sync.dma_start(wq_s[:, :, 0:256], wq[:, :, 0:256])
            nc.sync.dma_start(tq[:, :, 0:256], xq[:, :, 0:256])
            nc.sync.dma_start(wk_s[:, :, 0:256], wk[:, :, 0:256])
            nc.sync.dma_start(tk[:, :, 0:256], xk[:, :, 0:256])
            nc.sync.dma_start(bq_s[:, :], bq[:, :])
            nc.sync.dma_start(bk_s[:, :], bk[:, :])
            nc.sync.dma_start(wq_s[:, :, 256:512], wq[:, :, 256:512])
            nc.sync.dma_start(tq[:, :, 256:512], xq[:, :, 256:512])
            nc.sync.dma_start(wk_s[:, :, 256:512], wk[:, :, 256:512])
            nc.sync.dma_start(tk[:, :, 256:512], xk[:, :, 256:512])
            xq_tiles[0] = tq
            xk_tiles[0] = tk
            nc.sync.dma_start(wv_s[:, :, :], wv[:, :, :])
            t0 = xvp.tile([128, NCH, 512], BF16, tag="xv", name="xv0")
            nc.sync.dma_start(t0[:, :, :], xv[:, :, 0:512])
            xv_tiles[0] = t0
            vb_bcast = bass.AP(tensor=vb.tensor, offset=vb.offset,
                               ap=[[0, 128], [1, AUGW]])
            nc.gpsimd.dma_start(vb_s[:, :], vb_bcast)
            nc.sync.dma_start(msk_s[:, :], msk[:, :])
            nc.sync.dma_start(idn_s[:, :], idn[:, :])
            nc.sync.dma_start(wo_s[:, :, :], wo[:, :, :])

            qt_tiles = {}

            # ---- phase A unit generator: fp8 DoubleRow projections ----
            def a_units(n):
                qt_t = qtp.tile([128, 4, GW], F32R, tag="qt", name=f"qt{n}")
                qt_tiles[n] = qt_t

                def qk_unit(hf, g, x_of, w_s, b_s, is_q):
                    # split into two ~0.9us halves (one mi each) so fillers
                    # can weave between S/exp steps without starving ACT
                    ps_box = []

                    def half(mi):
                        def emit():
                            x_t = x_of[n]
                            if mi == 0:
                                ps_box.append(psA.tile(
                                    [128, 2, 256], F32, tag="pa",
                                    name=f"pa{n}_{hf}_{g}"))
                            ps = ps_box[0]
                            for c in range(NCH):
                                # start=True zeroes the whole psum bank:
                                # only the first write into the tile sets it
                                nc.tensor.matmul(
                                    ps[:, mi, :],
                                    w_s[:, c, (2 * g + mi) * 128:
                                        (2 * g + mi + 1) * 128],
                                    x_t[:, c, hf * 256:hf * 256 + 256],
                                    start=(c == 0 and mi == 0),
                                    stop=(c == NCH - 1),
                                    skip_group_check=True)
                            m = 2 * g + mi
                            if is_q:
                                dst = qt_t[:, m, hf * 256:hf * 256 + 256]
                            else:
                                dst = kt_s[:, m, n * 512 + hf * 256:
                                           n * 512 + hf * 256 + 256]
                            if BIASACT:
                                nc.scalar.activation(
                                    dst, ps[:, mi, :],
                                    func=mybir.ActivationFunctionType.Identity,
                                    bias=b_s[:, m:m + 1])
                            else:
                                nc.vector.tensor_scalar_add(
                                    dst, ps[:, mi, :], b_s[:, m:m + 1])
                        return emit
                    return [half(0), half(1)]

                def v_unit(tt, vhf):
                    def emit():
                        ps = psA.tile([128, 260], F32, tag="pa",
                                      name=f"pv{n}_{tt}_{vhf}")
                        xv_t = xv_tiles[n]
                        for c in range(NCH):
                            nc.tensor.matmul(
                                ps[:, :],
                                xv_t[:, c, tt * 128:(tt + 1) * 128],
                                wv_s[:, c, vhf * 260:(vhf + 1) * 260],
                                start=(c == 0), stop=(c == NCH - 1))
                        nc.vector.tensor_add(
                            vaug_s[:, n * 4 + tt, vhf * 260:(vhf + 1) * 260],
                            ps[:, :], vb_s[:, vhf * 260:(vhf + 1) * 260])
                    return emit

                units = []
                for hf in range(2):
                    for g in range(2):
                        units.extend(qk_unit(hf, g, xq_tiles, wq_s, bq_s, True))
                        units.extend(qk_unit(hf, g, xk_tiles, wk_s, bk_s,
                                             False))
                for hf in range(2):
                    for tt in (2 * hf, 2 * hf + 1):
                        for vhf in range(2):
                            units.append(v_unit(tt, vhf))
                return units

            ctxn_tiles = {}
            ctxT_tiles = {}

            # ---- phase B: one head of slice n ----
            # returns (s_emitters, av_emitters, finalize) so the slice loop
            # can weave the next head's first S blocks before this head's
            # tail, keeping ACT fed across head boundaries
            def plan_head(n, h):
                po = (h % 2) * 64
                mc = h // 2
                qt_t = qt_tiles[n]
                nkb = 4 * n + 4
                avps = psAV.tile([128, 4, DK + 1], F32, tag="av",
                                 name=f"av{n}_{h}")
                # S/exp units: full-history kb pairs, then 2 diag pairs
                pt_of = {}   # kb -> (tile, region)
                ptm_of = {}  # kb -> masked diag tile
                sunits = [("pair", p) for p in range(2 * n)]
                sunits += [("dpair", 0), ("dpair", 1)]

                def emit_s(u):
                    kind, a = u
                    sp = psS.tile([128, 2, 512], F32, tag="sp",
                                  name=f"sp{n}_{h}_{kind}{a}")
                    if kind == "pair":
                        for i in range(2):
                            kb = 2 * a + i
                            # regions 0/1 are in different banks: each needs
                            # its own start=True (bank-granular zeroing)
                            nc.tensor.matmul(
                                sp[:, i, :],
                                kt_s[po:po + 64, mc, kb * 128:(kb + 1) * 128],
                                qt_t[po:po + 64, mc, :],
                                start=True, stop=True,
                                skip_group_check=True)
                        pt = ptp.tile([128, 2, 512], BF16, tag="pt",
                                      name=f"pt{n}_{h}_p{a}")
                        nc.scalar.activation(
                            pt[:, :, :], sp[:, :, :],
                            func=mybir.ActivationFunctionType.Exp, scale=ESC)
                        pt_of[2 * a] = (pt, 0)
                        pt_of[2 * a + 1] = (pt, 1)
                    else:
                        # diagonal pair d: blocks jj = 2d, 2d+1 share one
                        # 2-bank psum tile and ONE exp; unwritten psum cols
                        # are zeroed by start=True (exp(0*s)=1, never read)
                        d = a
                        col0x = 256 * d
                        for i in range(2):
                            kb = 4 * n + 2 * d + i
                            jj = 2 * d + i
                            col0s = min(jj * 128, 256)
                            nc.tensor.matmul(
                                sp[:, i, col0s:],
                                kt_s[po:po + 64, mc, kb * 128:(kb + 1) * 128],
                                qt_t[po:po + 64, mc, col0s:],
                                start=True, stop=True, skip_group_check=True)
                        pt = ptp.tile([128, 2, 512], BF16, tag="pt",
                                      name=f"pt{n}_{h}_dp{d}")
                        nc.scalar.activation(
                            pt[:, :, col0x:], sp[:, :, col0x:],
                            func=mybir.ActivationFunctionType.Exp, scale=ESC)
                        for i in range(2):
                            jj = 2 * d + i
                            # out-of-place mask: unmasked consumers (j > jj)
                            # read pt directly without waiting on DVE
                            ptm = ptp.tile([128, 128], BF16, tag="ptm",
                                           name=f"ptm{n}_{h}_{2 * d + i}",
                                           bufs=4)
                            nc.vector.tensor_mul(
                                ptm[:, :],
                                pt[:, i, jj * 128:(jj + 1) * 128], msk_s[:, :])
                            pt_of[4 * n + jj] = (pt, i)
                            ptm_of[4 * n + jj] = ptm

                def emit_av(u):
                    kind, a = u
                    if kind == "pair":
                        kbs = [2 * a, 2 * a + 1]
                    else:
                        kbs = [4 * n + 2 * a, 4 * n + 2 * a + 1]
                    for kb in kbs:
                        j0 = max(0, kb - 4 * n)
                        pt, reg = pt_of[kb]
                        for j in range(j0, 4):
                            if j == kb - 4 * n:
                                lhs = ptm_of[kb][:, :]
                            else:
                                lhs = pt[:, reg, j * 128:(j + 1) * 128]
                            # whole-bank zero on start: only first mm sets it
                            nc.tensor.matmul(
                                avps[:, j, :], lhs,
                                vaug_s[:, kb, h * 65:(h + 1) * 65],
                                start=(kb == 0 and j == 0),
                                stop=(kb == 4 * n + j),
                                skip_group_check=True)

                def finalize():
                    rc = rcp.tile([128, 4], F32, tag="rc", name=f"rc{n}_{h}")
                    nc.vector.reciprocal(rc[:, :], avps[:, :, 64])
                    for j in range(4):
                        nc.vector.tensor_scalar_mul(
                            ctxn_tiles[(n, j)][:, h * 64:(h + 1) * 64],
                            avps[:, j, 0:64], rc[:, j:j + 1])

                s_emit = [(lambda u: (lambda: emit_s(u)))(u) for u in sunits]
                av_emit = [(lambda u: (lambda: emit_av(u)))(u) for u in sunits]
                return s_emit, av_emit, finalize

            # ---- phase C unit: token tile t, output half n2 ----
            out_tiles = {}

            def c_unit(n, j, n2):
                t = 4 * n + j
                # C units share the psA ring (A units are gone or sparse
                # by the time C drains); psC's bank went to psAV=2 which
                # unserializes consecutive heads' AV accumulation
                pool = psA
                ptag = "pa"

                def emit():
                    if n2 == 0 and n < NSL - 1:
                        out_tiles[t] = outsp.tile([128, D], F32, tag="outs",
                                                  name=f"out{t}")
                    cps = pool.tile([128, 512], F32, tag=ptag,
                                    name=f"cps{t}_{n2}")
                    ctxT_t = ctxT_tiles[(n, j)]
                    for c in range(4):
                        nc.tensor.matmul(
                            cps[:, :], ctxT_t[:, c, :],
                            wo_s[:, c, n2 * 512:(n2 + 1) * 512],
                            start=(c == 0), stop=(c == 3))
                    if n == NSL - 1:
                        # tail: per-half copy + immediate DMA (faster outs
                        # ring recycle than full-row tiles)
                        oh = outsp.tile([128, 512], F32, tag="outs",
                                        name=f"outh{t}_{n2}")
                        nc.vector.tensor_copy(oh[:, :], cps[:, :])
                        nc.sync.dma_start(
                            outp[t * 128:(t + 1) * 128,
                                 n2 * 512:(n2 + 1) * 512], oh[:, :])
                    else:
                        nc.vector.tensor_copy(
                            out_tiles[t][:, n2 * 512:(n2 + 1) * 512],
                            cps[:, :])
                        if n2 == 1:
                            nc.sync.dma_start(
                                outp[t * 128:(t + 1) * 128, :],
                                out_tiles[t][:, :])
                return emit

            # ---- main schedule ----
            for u in a_units(0):
                u()

            a_queue = []          # (slice, unit) in slice order
            carry = []            # next slice's pre-planned heads
            pending_c = []
            c_rate = dict(enumerate(CRATES))

            for n in range(NSL):
                if LOOKAHEAD:
                    if n == 0:
                        issue_xin(1)
                        issue_xin(2)
                        for u in a_units(1):
                            a_queue.append((1, u))
                        for u in a_units(2):
                            a_queue.append((2, u))
                    elif n == 1:
                        issue_xin(3)
                        for u in a_units(3):
                            a_queue.append((3, u))
                else:
                    if n < NSL - 1:
                        issue_xin(n + 1)
                        for u in a_units(n + 1):
                            a_queue.append((n + 1, u))
                for j in range(4):
                    ctxn_tiles[(n, j)] = ctxnp.tile(
                        [128, GW], BF16, tag="ctxn", name=f"ctxn{n}_{j}")
                fillq = []
                for _ in range(ARATE * HD):
                    if a_queue:
                        fillq.append(a_queue.pop(0)[1])
                for _ in range(c_rate.get(n, 2) * HD):
                    if pending_c:
                        fillq.append(pending_c.pop(0))
                if WEAVE:
                    prev_fin = None
                    for h in range(HD):
                        s_emit, av_emit, fin = plan_head(n, h)
                        ns_ = len(s_emit)
                        s_emit[0]()
                        if prev_fin is not None:
                            prev_fin()
                        s_emit[1]()
                        if fillq:
                            fillq.pop(0)()
                        for i in range(2, ns_):
                            s_emit[i]()
                            av_emit[i - 2]()
                            if fillq:
                                fillq.pop(0)()
                        av_emit[ns_ - 2]()
                        av_emit[ns_ - 1]()
                        prev_fin = fin
                    prev_fin()
                    while fillq:
                        fillq.pop(0)()
                else:
                    nfill = len(fillq)
                    for h in range(HD):
                        if h < len(carry):
                            s_emit, av_emit, fin = carry[h]
                        else:
                            s_emit, av_emit, fin = plan_head(n, h)
                            s_emit[0]()
                            if len(s_emit) > 1:
                                s_emit[1]()
                        ns_ = len(s_emit)
                        for i in range(2, ns_):
                            s_emit[i]()
                            av_emit[i - 2]()
                        av_emit[ns_ - 2]()
                        av_emit[ns_ - 1]()
                        fin()
                        take = (nfill * (h + 1)) // HD - (nfill * h) // HD
                        for _ in range(take):
                            if fillq:
                                fillq.pop(0)()
                    # pre-emit next slice's h0/h1 S prologues BEFORE the
                    # leftover A-drain (V halves): they only need the Q/K
                    # units already drained, so ACT stays fed across the
                    # slice boundary
                    carry = []
                    if n < NSL - 1:
                        # interleave the leftover V-unit drain between the
                        # two pre-emitted prologues: keeps ACT fed while
                        # getting V_aug moving for the next slice's AVs
                        for hh in range(2):
                            nxt = plan_head(n + 1, hh)
                            nxt[0][0]()
                            nxt[0][1]()
                            carry.append(nxt)
                            for _ in range(2):
                                if a_queue and a_queue[0][0] <= n + 1:
                                    a_queue.pop(0)[1]()
                # B(n+1) needs all of A(n+1) done
                while a_queue and a_queue[0][0] <= n + 1:
                    a_queue.pop(0)[1]()
                if n < NSL - 1:
                    for j in range(4):
                        ct = ctxTp.tile([128, 4, 128], BF16, tag="ctxT",
                                        name=f"ctxT{n}_{j}")
                        nc.sync.dma_start_transpose(ct,
                                                    ctxn_tiles[(n, j)][:, :])
                        ctxT_tiles[(n, j)] = ct
                    for j in range(4):
                        for n2 in range(2):
                            pending_c.append(c_unit(n, j, n2))
                else:
                    # tail: PE transposes (PE is idle here) + DVE copy cut
                    # the ~2.8us-per-tile xbar DMA latency off the critical
                    # path; interleave each transpose with its C unit
                    for j in range(4):
                        tp = psA.tile([128, 4, 128], BF16, tag="pa",
                                      name=f"tp{j}")
                        cn_t = ctxn_tiles[(n, j)]
                        for c in range(4):
                            nc.tensor.transpose(tp[:, c, :],
                                                cn_t[:, c * 128:(c + 1) * 128],
                                                idn_s[:, :])
                        ct = ctxTp.tile([128, 4, 128], BF16, tag="ctxT",
                                        name=f"ctxT{n}_{j}")
                        nc.vector.tensor_copy(ct[:, :, :], tp[:, :, :])
                        ctxT_tiles[(n, j)] = ct
                        pending_c.append(c_unit(n, j, 0))
                        pending_c.append(c_unit(n, j, 1))
            while pending_c:
                pending_c.pop(0)()

            if dbg:
                nc.sync.dma_start(qt_dbg[:, :, :],
                                  qt_tiles[0][:, :, :].bitcast(F32))
                nc.sync.dma_start(kt_dbg[:, :, :], kt_s[:, :, :].bitcast(F32))
                nc.sync.dma_start(vg_dbg[:, :, :], vaug_s[:, :, :])
                for n in range(NSL):
                    for j in range(4):
                        nc.sync.dma_start(cn_dbg[:, 4 * n + j, :],
                                          ctxn_tiles[(n, j)][:, :])
                        nc.sync.dma_start(ct_dbg[:, 4 * n + j, :, :],
                                          ctxT_tiles[(n, j)][:, :, :])

    nc.compile()
    return nc


_NC = None
LAST_RESULTS = None


def kernel(**inputs):
    global _NC, LAST_RESULTS
    import os
    if _NC is None:
        _NC = _build_nc()

    f = lambda a: np.asarray(a, dtype=np.float32)
    q, k, v = f(inputs["q"]), f(inputs["k"]), f(inputs["v"])
    wq_w, wq_b = f(inputs["wq_w"]), f(inputs["wq_b"])
    wk_w, wk_b = f(inputs["wk_w"]), f(inputs["wk_b"])
    wv_w, wv_b = f(inputs["wv_w"]), f(inputs["wv_b"])
    wo_w, wo_b = f(inputs["wo_w"]), f(inputs["wo_b"])

    bf = ml_dtypes.bfloat16
    f8 = ml_dtypes.float8_e4m3

    def chunk_rows(a, inner):
        # [1024, X] -> [128, 8, X] with row r = c*128+p -> [p, c, :]
        return np.ascontiguousarray(
            a.reshape(NCH, 128, inner).transpose(1, 0, 2))

    msk = np.ascontiguousarray(
        (np.arange(128)[None, :] >= np.arange(128)[:, None])).astype(bf)

    gmaps = []
    for g in range(2):
        sl = slice(g * GW, (g + 1) * GW)
        wqT = chunk_rows(wq_w[sl].T, GW).astype(bf)
        wkT = chunk_rows(wk_w[sl].T, GW).astype(bf)
        wvT = np.zeros((D, AUGW), np.float32)
        vbias = np.zeros((AUGW,), np.float32)
        for h in range(HD):
            wvT[:, h * 65:h * 65 + 64] = wv_w[g * GW + h * 64:
                                              g * GW + (h + 1) * 64].T * WS
            vbias[h * 65:h * 65 + 64] = wv_b[g * GW + h * 64:
                                             g * GW + (h + 1) * 64] * WS
            vbias[h * 65 + 64] = WS
        woT = np.ascontiguousarray(
            wo_w[:, sl].T.reshape(4, 128, D).transpose(1, 0, 2)).astype(bf)
        bqT = np.ascontiguousarray(wq_b[sl].reshape(4, 128).T)
        bkT = np.ascontiguousarray(wk_b[sl].reshape(4, 128).T)
        gmaps.append(dict(wq=wqT, wk=wkT, wv=chunk_rows(wvT, AUGW).astype(bf),
                          wo=woT, bq=bqT, bk=bkT, vb=vbias, msk=msk,
                          idn=np.eye(128, dtype=np.float32).astype(bf)))

    bmaps = []
    for b in range(B):
        bmaps.append(dict(
            xq=chunk_rows(np.ascontiguousarray(q[b].T), L).astype(bf),
            xk=chunk_rows(np.ascontiguousarray(k[b].T), L).astype(bf),
            xv=chunk_rows(np.ascontiguousarray(v[b].T), L).astype(bf)))

    in_maps = [dict(**bmaps[c // 2], **gmaps[c % 2]) for c in range(8)]

    trace = bool(int(os.environ.get("KERNEL_TRACE", "0")))
    res = run_bass_kernel_spmd(_NC, in_maps, list(range(8)), trace=trace)
    LAST_RESULTS = res

    out = np.empty((B, L, D), np.float32)
    for b in range(B):
        out[b] = (res.results[2 * b]["outp"] + res.results[2 * b + 1]["outp"]
                  + wo_b[None, :])
    return out
